# revision 20
# baseline (speedup 1.0000x reference)
"""GATv2 (2-layer) fully fused on 8 Trainium2 NeuronCores.

Design (dst-range edge sharding):
  - Nodes sharded 12500/core (padded 12544). Edges (incl. mean-fill self
    loops) sorted by dst and assigned to the core owning dst.
  - Per core: dense transforms xl/xr = x @ Wl|Wr + b on PE; AllGather of the
    per-core xl shards builds a full local xl table in each core's HBM;
    edge phase gathers xl[src] (indirect DMA from the gathered table) and
    xr[dst] (indirect DMA from the local xr table), computes GATv2 scores,
    and segment-softmax-aggregates via indicator matmuls into PSUM.
  - Softmax skips the segment-max subtraction: logits are O(30) so exp stays
    comfortably inside fp32 range, and out = (sum p*xl)/(sum p) is exact.
  - Edges are host-packed into chunks of 128; each group of 128 dst nodes
    owns CG chunks (padded with dummy edges, dstg=200 -> zero indicator row).
  - All per-core inputs ship as ONE packed [128, TOT] f32 tensor (int32
    index columns bitcast) so every consumer waits on a single DMA lane
    (walrus allows only one sync wait on a Matmult).

kernel(**inputs) -> [100000, 64] fp32.
"""
import os
import numpy as np
import jax
from jax.sharding import Mesh, PartitionSpec, NamedSharding
from jax.experimental.shard_map import shard_map

import concourse.bacc as bacc
import concourse.tile as tile
from concourse import mybir, bass
from concourse.bass2jax import (_bass_exec_p, install_neuronx_cc_hook,
                                partition_id_tensor)

F32 = mybir.dt.float32
BF16 = mybir.dt.bfloat16
I32 = mybir.dt.int32
AF = mybir.ActivationFunctionType
ALU = mybir.AluOpType

N = 100000
IN = 128
HC = 64
NCORES = 8
PER = N // NCORES            # 12500
PERPAD = 12544               # 98 * 128
G = PERPAD // 128            # 98 groups/core
NEG = 0.2

# packed-input column offsets (CG-independent part)
OF_XT = 0
OF_CST = 12544
CSTW = 1024
OF_W1 = OF_CST + CSTW
OF_W2 = OF_W1 + 128           # W2 in bf16: 64 f32 cols
OF_IDX = OF_W2 + 64           # then srcg|dstg|wcol each NCH wide
# cst sub-offsets (relative to OF_CST); *B = bf16 packed into f32 cols
B1, B2 = 0, 128
ATT1, ATT2 = 256, 320
BIA1, BIA2 = 384, 448
WE1, WE2 = 512, 576
IOTA, IDEN = 640, 768
ATT1B, ATT2B, IDEN16 = 896, 928, 960

_cache = {}
_PHASE = int(os.environ.get("K2_PHASE", "4"))


# ----------------------------------------------------------------- builder
def _build(CG, phase=4, abl=frozenset()):
    NCH = G * CG
    TOT = OF_IDX + 3 * NCH
    nc = bacc.Bacc("TRN2", target_bir_lowering=False, debug=False)
    t_mega = nc.dram_tensor("mega", [128, TOT], F32, kind="ExternalInput")
    t_out = nc.dram_tensor("out", [PERPAD, 64], F32, kind="ExternalOutput")

    with tile.TileContext(nc) as tc:
        with tc.tile_pool(name="dram", bufs=1, space="DRAM") as dpool, \
             tc.tile_pool(name="big", bufs=1) as bigp, \
             tc.tile_pool(name="sb", bufs=2) as pool, \
             tc.tile_pool(name="gat", bufs=2) as gpool, \
             tc.tile_pool(name="pd", bufs=2, space="PSUM") as psd, \
             tc.tile_pool(name="pt", bufs=2, space="PSUM") as pst, \
             tc.tile_pool(name="pz", bufs=1, space="PSUM") as psz, \
             tc.tile_pool(name="pu", bufs=2, space="PSUM") as psu:

            xl1_sh = dpool.tile([PERPAD, 64], BF16)
            xr1_tab = dpool.tile([PERPAD, 64], BF16)
            xl1_tab = dpool.tile([NCORES * PERPAD, 64], BF16, addr_space="Shared")
            xl2_sh = dpool.tile([PERPAD, 64], BF16)
            xr2_tab = dpool.tile([PERPAD, 64], BF16)
            xl2_tab = dpool.tile([NCORES * PERPAD, 64], BF16, addr_space="Shared")

            mega = bigp.tile([128, TOT], F32)
            nc.sync.dma_start(out=mega[:], in_=t_mega[:])
            xT = mega[:, OF_XT:OF_XT + PERPAD]
            cst = mega[:, OF_CST:OF_CST + CSTW]
            att1b = cst[:, ATT1B:ATT1B + 32].bitcast(BF16)
            att2b = cst[:, ATT2B:ATT2B + 32].bitcast(BF16)
            iden16 = cst[:, IDEN16:IDEN16 + 64].bitcast(BF16)
            W1 = mega[:, OF_W1:OF_W1 + 128]
            W2 = mega[:, OF_W2:OF_W2 + 64].bitcast(BF16)
            srcg = mega[:, OF_IDX + 0 * NCH:OF_IDX + 1 * NCH].bitcast(I32)
            dstg = mega[:, OF_IDX + 1 * NCH:OF_IDX + 2 * NCH]
            wcol = mega[:, OF_IDX + 2 * NCH:OF_IDX + 3 * NCH]
            hT = bigp.tile([128, G * 64], BF16)   # tile t -> part 64*(t%2), col (t//2)*128

            # ---------------- dense 1
            for t in range(G):
                pd = psd.tile([128, 128], F32, space="PSUM", tag="pd")
                nc.tensor.matmul(pd[:], lhsT=xT[:, t * 128:(t + 1) * 128],
                                 rhs=W1[:], start=True, stop=True)
                xlr = pool.tile([128, 128], BF16, tag="xlr")
                nc.vector.tensor_add(xlr[:], pd[:], cst[:, B1:B1 + 128])
                nc.sync.dma_start(out=xl1_sh[t * 128:(t + 1) * 128, :], in_=xlr[:, 0:64])
                nc.sync.dma_start(out=xr1_tab[t * 128:(t + 1) * 128, :], in_=xlr[:, 64:128])

            nc.gpsimd.collective_compute(
                "AllGather", ALU.bypass,
                replica_groups=[list(range(NCORES))],
                ins=[xl1_sh[:]], outs=[xl1_tab[:]])

            if phase == 1:
                for t in range(G):
                    ot = pool.tile([128, 64], F32, tag="otp1")
                    nc.sync.dma_start(out=ot[:], in_=xl1_tab[t * 128:(t + 1) * 128, :])
                    nc.sync.dma_start(out=t_out[t * 128:(t + 1) * 128, :], in_=ot[:])

            # ---------------- edge phase 1 (heads=2, c=32)
            CGW = CG * 64
            for g in range(G if phase >= 2 else 0):
                pu = psu.tile([128, 66], F32, space="PSUM", tag="pu")
                xl_G = gpool.tile([128, CGW], BF16, tag="xl", bufs=3)
                for j in range(CG):
                    c = g * CG + j
                    nc.gpsimd.indirect_dma_start(
                        out=xl_G[:, j * 64:(j + 1) * 64], out_offset=None,
                        in_=xl1_tab[:],
                        in_offset=bass.IndirectOffsetOnAxis(ap=srcg[:, c:c + 1], axis=0))
                xr_grp = gpool.tile([128, 64], BF16, tag="xr")
                nc.sync.dma_start(out=xr_grp[:], in_=xr1_tab[g * 128:(g + 1) * 128, :])
                # z0 = xl + We1*w  (group-wide)
                t1G = pool.tile([128, CGW], BF16, tag="t1")
                nc.vector.tensor_tensor(
                    out=t1G.rearrange("p (j c) -> p j c", c=64),
                    in0=cst[:, WE1:WE1 + 64].rearrange("p (o c) -> p o c", o=1)
                        .to_broadcast([128, CG, 64]),
                    in1=wcol[:, g * CG:(g + 1) * CG].to_broadcast([128, CG, 64]),
                    op=ALU.mult)
                z0G = pool.tile([128, CGW], BF16, tag="z0")
                nc.vector.tensor_add(z0G[:], xl_G[:], t1G[:])
                # all CG indicators in ONE is_equal
                indG = pool.tile([128, CG * 128], BF16, tag="ind")
                nc.vector.tensor_tensor(
                    out=indG.rearrange("p (j d) -> p j d", d=128),
                    in0=dstg[:, g * CG:(g + 1) * CG].to_broadcast([128, CG, 128]),
                    in1=cst[:, IOTA:IOTA + 128].rearrange("p (o d) -> p o d", o=1)
                        .to_broadcast([128, CG, 128]),
                    op=ALU.is_equal)
                pzG = psz.tile([128, CGW], F32, space="PSUM", tag="pz")
                indTG = pool.tile([128, CG * 128], BF16, tag="indT")
                for j in range(CG):
                    ptt = pst.tile([128, 128], BF16, space="PSUM", tag="ptt")
                    nc.tensor.transpose(out=ptt[:], in_=indG[:, j * 128:(j + 1) * 128],
                                        identity=iden16[:])
                    nc.vector.tensor_copy(indTG[:, j * 128:(j + 1) * 128], ptt[:])
                    nc.tensor.matmul(pzG[:, j * 64:(j + 1) * 64],
                                     lhsT=indTG[:, j * 128:(j + 1) * 128],
                                     rhs=xr_grp[:], start=True, stop=True)
                pzB = pool.tile([128, CGW], BF16, tag="pzb")
                nc.vector.tensor_copy(pzB[:], pzG[:])
                zG = pool.tile([128, CGW], BF16, tag="z")
                nc.vector.tensor_add(zG[:], z0G[:], pzB[:])
                lrG = pool.tile([128, CGW], BF16, tag="lr")
                nc.scalar.activation(lrG[:], zG[:], AF.Prelu, alpha=NEG)
                lrwG = pool.tile([128, CGW], BF16, tag="lrw")
                nc.vector.tensor_tensor(
                    out=lrwG.rearrange("p (j c) -> p j c", c=64),
                    in0=lrG.rearrange("p (j c) -> p j c", c=64),
                    in1=att1b.rearrange("p (o c) -> p o c", o=1)
                        .to_broadcast([128, CG, 64]),
                    op=ALU.mult)
                laG = pool.tile([128, CG * 2], F32, tag="la")
                nc.vector.tensor_reduce(
                    out=laG[:], in_=lrwG.rearrange("p (a c) -> p a c", c=32),
                    axis=mybir.AxisListType.X, op=ALU.add)
                vtG = pool.tile([128, CG * 66], BF16, tag="vt")
                vt3 = vtG.rearrange("p (j k) -> p j k", k=66)
                nc.scalar.activation(vt3[:, :, 64:66],
                                     laG.rearrange("p (j h) -> p j h", h=2), AF.Exp)
                xl3 = xl_G.rearrange("p (j c) -> p j c", c=64)
                for h in range(2):
                    nc.vector.tensor_tensor(
                        out=vt3[:, :, h * 32:(h + 1) * 32],
                        in0=xl3[:, :, h * 32:(h + 1) * 32],
                        in1=vt3[:, :, 64 + h:65 + h].to_broadcast([128, CG, 32]),
                        op=ALU.mult)
                for j in range(CG):
                    nc.tensor.matmul(pu[:], lhsT=indG[:, j * 128:(j + 1) * 128],
                                     rhs=vtG[:, j * 66:(j + 1) * 66],
                                     start=(j == 0), stop=(j == CG - 1))
                # finalize group: h = relu(u/s + bias1), store transposed
                # (clamp s away from 0 so empty padding rows give 0, not NaN)
                sm = pool.tile([128, 2], F32, tag="sm")
                nc.vector.tensor_scalar_max(sm[:], pu[:, 64:66], 1e-30)
                rec = pool.tile([128, 2], F32, tag="rec")
                nc.vector.reciprocal(rec[:], sm[:])
                h = pool.tile([128, 64], F32, tag="h")
                nc.vector.tensor_mul(h[:, 0:32], pu[:, 0:32],
                                     rec[:, 0:1].to_broadcast([128, 32]))
                nc.vector.tensor_mul(h[:, 32:64], pu[:, 32:64],
                                     rec[:, 1:2].to_broadcast([128, 32]))
                nc.vector.tensor_add(h[:], h[:], cst[:, BIA1:BIA1 + 64])
                hr = pool.tile([128, 64], BF16, tag="hr")
                nc.vector.tensor_scalar_max(hr[:], h[:], 0.0)
                ptt = pst.tile([64, 128], BF16, space="PSUM", tag="ptt")
                nc.tensor.transpose(out=ptt[:], in_=hr[:], identity=iden16[:])
                po = 64 * (g % 2)
                nc.vector.tensor_copy(hT[po:po + 64, (g // 2) * 128:(g // 2) * 128 + 128],
                                      ptt[:])
                if phase == 2:
                    nc.sync.dma_start(out=t_out[g * 128:(g + 1) * 128, :], in_=hr[:])

            # ---------------- dense 2
            for t in range(G if phase >= 3 else 0):
                po = 64 * (t % 2)
                pd = psd.tile([128, 128], F32, space="PSUM", tag="pd")
                nc.tensor.matmul(pd[:], lhsT=hT[po:po + 64, (t // 2) * 128:(t // 2) * 128 + 128],
                                 rhs=W2[po:po + 64, :], start=True, stop=True)
                xlr = pool.tile([128, 128], BF16, tag="xlr")
                nc.vector.tensor_add(xlr[:], pd[:], cst[:, B2:B2 + 128])
                nc.sync.dma_start(out=xl2_sh[t * 128:(t + 1) * 128, :], in_=xlr[:, 0:64])
                nc.sync.dma_start(out=xr2_tab[t * 128:(t + 1) * 128, :], in_=xlr[:, 64:128])

            if phase == 31:
                for t in range(G):
                    ot = pool.tile([128, 64], F32, tag="otp3")
                    nc.sync.dma_start(out=ot[:], in_=xl2_sh[t * 128:(t + 1) * 128, :])
                    nc.sync.dma_start(out=t_out[t * 128:(t + 1) * 128, :], in_=ot[:])

            if phase >= 3 and phase != 31:
                nc.gpsimd.collective_compute(
                    "AllGather", ALU.bypass,
                    replica_groups=[list(range(NCORES))],
                    ins=[xl2_sh[:]], outs=[xl2_tab[:]])

            if phase == 3:
                for t in range(G):
                    ot = pool.tile([128, 64], F32, tag="otp3")
                    nc.sync.dma_start(out=ot[:], in_=xl2_tab[t * 128:(t + 1) * 128, :])
                    nc.sync.dma_start(out=t_out[t * 128:(t + 1) * 128, :], in_=ot[:])

            # ---------------- edge phase 2 (heads=1, c=64)
            for g in range(G if phase >= 4 and phase != 31 else 0):
                pu = psu.tile([128, 65], F32, space="PSUM", tag="pu")
                xl_G = gpool.tile([128, CGW], BF16, tag="xl", bufs=3)
                for j in range(CG):
                    c = g * CG + j
                    nc.gpsimd.indirect_dma_start(
                        out=xl_G[:, j * 64:(j + 1) * 64], out_offset=None,
                        in_=xl2_tab[:],
                        in_offset=bass.IndirectOffsetOnAxis(ap=srcg[:, c:c + 1], axis=0))
                xr_grp = gpool.tile([128, 64], BF16, tag="xr")
                nc.sync.dma_start(out=xr_grp[:], in_=xr2_tab[g * 128:(g + 1) * 128, :])
                t1G = pool.tile([128, CGW], BF16, tag="t1")
                nc.vector.tensor_tensor(
                    out=t1G.rearrange("p (j c) -> p j c", c=64),
                    in0=cst[:, WE2:WE2 + 64].rearrange("p (o c) -> p o c", o=1)
                        .to_broadcast([128, CG, 64]),
                    in1=wcol[:, g * CG:(g + 1) * CG].to_broadcast([128, CG, 64]),
                    op=ALU.mult)
                z0G = pool.tile([128, CGW], BF16, tag="z0")
                nc.vector.tensor_add(z0G[:], xl_G[:], t1G[:])
                indG = pool.tile([128, CG * 128], BF16, tag="ind")
                nc.vector.tensor_tensor(
                    out=indG.rearrange("p (j d) -> p j d", d=128),
                    in0=dstg[:, g * CG:(g + 1) * CG].to_broadcast([128, CG, 128]),
                    in1=cst[:, IOTA:IOTA + 128].rearrange("p (o d) -> p o d", o=1)
                        .to_broadcast([128, CG, 128]),
                    op=ALU.is_equal)
                pzG = psz.tile([128, CGW], F32, space="PSUM", tag="pz")
                indTG = pool.tile([128, CG * 128], BF16, tag="indT")
                for j in range(CG):
                    ptt = pst.tile([128, 128], BF16, space="PSUM", tag="ptt")
                    nc.tensor.transpose(out=ptt[:], in_=indG[:, j * 128:(j + 1) * 128],
                                        identity=iden16[:])
                    nc.vector.tensor_copy(indTG[:, j * 128:(j + 1) * 128], ptt[:])
                    nc.tensor.matmul(pzG[:, j * 64:(j + 1) * 64],
                                     lhsT=indTG[:, j * 128:(j + 1) * 128],
                                     rhs=xr_grp[:], start=True, stop=True)
                pzB = pool.tile([128, CGW], BF16, tag="pzb")
                nc.vector.tensor_copy(pzB[:], pzG[:])
                zG = pool.tile([128, CGW], BF16, tag="z")
                nc.vector.tensor_add(zG[:], z0G[:], pzB[:])
                lrG = pool.tile([128, CGW], BF16, tag="lr")
                nc.scalar.activation(lrG[:], zG[:], AF.Prelu, alpha=NEG)
                lrwG = pool.tile([128, CGW], BF16, tag="lrw")
                nc.vector.tensor_tensor(
                    out=lrwG.rearrange("p (j c) -> p j c", c=64),
                    in0=lrG.rearrange("p (j c) -> p j c", c=64),
                    in1=att2b.rearrange("p (o c) -> p o c", o=1)
                        .to_broadcast([128, CG, 64]),
                    op=ALU.mult)
                laG = pool.tile([128, CG], F32, tag="la2")
                nc.vector.tensor_reduce(
                    out=laG[:], in_=lrwG.rearrange("p (a c) -> p a c", c=64),
                    axis=mybir.AxisListType.X, op=ALU.add)
                vtG = pool.tile([128, CG * 65], BF16, tag="vt")
                vt3 = vtG.rearrange("p (j k) -> p j k", k=65)
                nc.scalar.activation(vt3[:, :, 64:65],
                                     laG.rearrange("p (j h) -> p j h", h=1), AF.Exp)
                xl3 = xl_G.rearrange("p (j c) -> p j c", c=64)
                nc.vector.tensor_tensor(
                    out=vt3[:, :, 0:64], in0=xl3[:, :, :],
                    in1=vt3[:, :, 64:65].to_broadcast([128, CG, 64]),
                    op=ALU.mult)
                for j in range(CG):
                    nc.tensor.matmul(pu[:], lhsT=indG[:, j * 128:(j + 1) * 128],
                                     rhs=vtG[:, j * 65:(j + 1) * 65],
                                     start=(j == 0), stop=(j == CG - 1))
                sm = pool.tile([128, 1], F32, tag="sm2")
                nc.vector.tensor_scalar_max(sm[:], pu[:, 64:65], 1e-30)
                rec = pool.tile([128, 1], F32, tag="rec2")
                nc.vector.reciprocal(rec[:], sm[:])
                o = pool.tile([128, 64], F32, tag="o")
                nc.vector.tensor_mul(o[:], pu[:, 0:64], rec[:].to_broadcast([128, 64]))
                nc.vector.tensor_add(o[:], o[:], cst[:, BIA2:BIA2 + 64])
                nc.sync.dma_start(out=t_out[g * 128:(g + 1) * 128, :], in_=o[:])
    nc.compile()
    return nc


# ----------------------------------------------------------------- runner
def _make_runner(nc):
    install_neuronx_cc_hook()
    in_names, out_names, out_avals = [], [], []
    partition_name = nc.partition_id_tensor.name if nc.partition_id_tensor else None
    for alloc in nc.m.functions[0].allocations:
        if not isinstance(alloc, mybir.MemoryLocationSet):
            continue
        name = alloc.memorylocations[0].name
        if alloc.kind == "ExternalInput":
            if name != partition_name:
                in_names.append(name)
        elif alloc.kind == "ExternalOutput":
            out_names.append(name)
            out_avals.append(jax.core.ShapedArray(tuple(alloc.tensor_shape),
                                                  mybir.dt.np(alloc.dtype)))
    n_params = len(in_names)
    n_outs = len(out_avals)
    all_in_names = list(in_names) + list(out_names)
    if partition_name is not None:
        all_in_names.append(partition_name)

    def _body(*args):
        operands = list(args)
        if partition_name is not None:
            operands.append(partition_id_tensor())
        outs = _bass_exec_p.bind(
            *operands,
            out_avals=tuple(out_avals),
            in_names=tuple(all_in_names),
            out_names=tuple(out_names),
            lowering_input_output_aliases=(),
            sim_require_finite=True,
            sim_require_nnan=True,
            nc=nc,
        )
        return tuple(outs)

    devices = jax.devices()[:NCORES]
    mesh = Mesh(np.asarray(devices), ("core",))
    in_specs = (PartitionSpec("core"),) * (n_params + n_outs)
    out_specs = (PartitionSpec("core"),) * n_outs
    fn = jax.jit(shard_map(_body, mesh=mesh, in_specs=in_specs,
                           out_specs=out_specs, check_rep=False),
                 keep_unused=True)
    return fn, in_names, out_names, out_avals, mesh, devices


def _stage(runner, in_maps):
    """Device-put per-core inputs (plus zero output feeds) as sharded arrays."""
    fn, in_names, out_names, out_avals, mesh, devices = runner
    staged = []
    for name in in_names:
        shards = [jax.device_put(np.ascontiguousarray(in_maps[k][name]), devices[k])
                  for k in range(NCORES)]
        jax.block_until_ready(shards)
        shp = in_maps[0][name].shape
        arr = jax.make_array_from_single_device_arrays(
            (NCORES * shp[0],) + tuple(shp[1:]),
            NamedSharding(mesh, PartitionSpec("core")), shards)
        staged.append(arr)
    for av in out_avals:
        z = np.zeros(av.shape, av.dtype)
        shards = [jax.device_put(z, d) for d in devices]
        jax.block_until_ready(shards)
        arr = jax.make_array_from_single_device_arrays(
            (NCORES * av.shape[0],) + tuple(av.shape[1:]),
            NamedSharding(mesh, PartitionSpec("core")), shards)
        staged.append(arr)
    return staged


# ----------------------------------------------------------------- host prep
def _host_prep(x, edge_index, edge_weight,
               W1l, b1l, W1r, b1r, We1, att1, bias1,
               W2l, b2l, W2r, b2r, We2, att2, bias2):
    src = edge_index[0].astype(np.int64)
    dst = edge_index[1].astype(np.int64)
    ew = edge_weight[:, 0].astype(np.float64)
    deg = np.bincount(dst, minlength=N).astype(np.float64)
    wsum = np.bincount(dst, weights=ew, minlength=N)
    loop_w = (wsum / np.maximum(deg, 1.0)).astype(np.float32)

    allsrc = np.concatenate([src, np.arange(N, dtype=np.int64)])
    alldst = np.concatenate([dst, np.arange(N, dtype=np.int64)])
    allw = np.concatenate([edge_weight[:, 0].astype(np.float32), loop_w])
    order = np.argsort(alldst, kind="stable")
    ss, ds, ws = allsrc[order], alldst[order], allw[order]

    core = ds // PER
    loc = ds % PER
    gid = core * G + loc // 128
    gcnt = np.bincount(gid, minlength=NCORES * G)
    CG = max(2, int(np.ceil(gcnt.max() / 128.0)))
    NCH = G * CG
    gstart = np.zeros(NCORES * G + 1, np.int64)
    np.cumsum(gcnt, out=gstart[1:])
    slot = np.arange(ds.shape[0], dtype=np.int64) - gstart[gid]

    SLOTS = NCH * 128
    srcg = np.zeros((NCORES, SLOTS), np.int32)
    dstl = np.zeros((NCORES, SLOTS), np.int32)
    dstg = np.full((NCORES, SLOTS), 200.0, np.float32)
    wpad = np.zeros((NCORES, SLOTS), np.float32)
    ce = gid // G
    pos = (gid % G) * (CG * 128) + slot
    srcg[ce, pos] = ((ss // PER) * PERPAD + ss % PER).astype(np.int32)
    dstl[ce, pos] = loc.astype(np.int32)
    dstg[ce, pos] = (loc % 128).astype(np.float32)
    wpad[ce, pos] = ws

    def colmaj(a):  # [SLOTS] -> [128, NCH]
        return np.ascontiguousarray(a.reshape(NCH, 128).T)

    import ml_dtypes
    bf = ml_dtypes.bfloat16
    cst = np.zeros((128, 1024), np.float32)
    cst[:, 0:128] = np.concatenate([b1l, b1r])[None, :]
    cst[:, 128:256] = np.concatenate([b2l, b2r])[None, :]
    cst[:, 256:320] = att1.reshape(-1)[None, :]
    cst[:, 320:384] = att2.reshape(-1)[None, :]
    cst[:, 384:448] = bias1[None, :]
    cst[:, 448:512] = bias2[None, :]
    cst[:, 512:576] = We1.reshape(-1)[None, :]
    cst[:, 576:640] = We2.reshape(-1)[None, :]
    cst[:, 640:768] = np.arange(128, dtype=np.float32)[None, :]
    cst[:, 768:896] = np.eye(128, dtype=np.float32)
    cst[:, 896:928] = np.ascontiguousarray(
        att1.reshape(-1).astype(bf)).view(np.float32)[None, :]
    cst[:, 928:960] = np.ascontiguousarray(
        att2.reshape(-1).astype(bf)).view(np.float32)[None, :]
    cst[:, 960:1024] = np.ascontiguousarray(np.eye(128, dtype=bf)).view(np.float32)

    W1lr = np.concatenate([W1l, W1r], axis=1)           # [128,128]
    W2cat = np.concatenate([W2l, W2r], axis=1)          # [64,128]
    W2lr = np.concatenate([W2cat, W2cat], axis=0)       # [128,128]

    TOT = OF_IDX + 3 * NCH
    in_maps = []
    for k in range(NCORES):
        mega = np.zeros((128, TOT), np.float32)
        mega[:, OF_XT + 0:OF_XT + PER] = x[k * PER:(k + 1) * PER].T
        mega[:, OF_CST:OF_CST + CSTW] = cst
        mega[:, OF_W1:OF_W1 + 128] = W1lr
        mega[:, OF_W2:OF_W2 + 64] = np.ascontiguousarray(
            W2lr.astype(bf)).view(np.float32)
        mega[:, OF_IDX + 0 * NCH:OF_IDX + 1 * NCH] = colmaj(srcg[k]).view(np.float32)
        mega[:, OF_IDX + 1 * NCH:OF_IDX + 2 * NCH] = colmaj(dstg[k])
        mega[:, OF_IDX + 2 * NCH:OF_IDX + 3 * NCH] = colmaj(wpad[k])
        in_maps.append(dict(mega=mega))
    return CG, in_maps


def _get_program(CG):
    key = ("fused", CG, _PHASE)
    if key not in _cache:
        nc = _build(CG, _PHASE)
        _cache[key] = (nc, _make_runner(nc))
    return _cache[key]


def _execute(runner, staged):
    fn = runner[0]
    out = fn(*staged)
    return out


def kernel(x, edge_index, edge_weight,
           W1l, b1l, W1r, b1r, We1, att1, bias1,
           W2l, b2l, W2r, b2r, We2, att2, bias2):
    f32 = lambda a: np.asarray(a, np.float32)
    CG, in_maps = _host_prep(
        f32(x), np.asarray(edge_index), f32(edge_weight),
        f32(W1l), f32(b1l), f32(W1r), f32(b1r), f32(We1), f32(att1), f32(bias1),
        f32(W2l), f32(b2l), f32(W2r), f32(b2r), f32(We2), f32(att2), f32(bias2))
    nc, runner = _get_program(CG)
    staged = _stage(runner, in_maps)
    out = _execute(runner, staged)
    glob = np.asarray(out[0])          # [8*PERPAD, 64]
    res = np.empty((N, 64), np.float32)
    for k in range(NCORES):
        res[k * PER:(k + 1) * PER] = glob[k * PERPAD:k * PERPAD + PER]
    return res



# revision 23
# speedup vs baseline: 1.0810x; 1.0810x over previous
"""GATv2 (2-layer) fully fused on 8 Trainium2 NeuronCores.

Design (dst-range edge sharding):
  - Nodes sharded 12500/core (padded 12544). Edges (incl. mean-fill self
    loops) sorted by dst and assigned to the core owning dst.
  - Per core: dense transforms xl/xr = x @ Wl|Wr + b on PE; AllGather of the
    per-core xl shards builds a full local xl table in each core's HBM;
    edge phase gathers xl[src] (indirect DMA from the gathered table) and
    xr[dst] (indirect DMA from the local xr table), computes GATv2 scores,
    and segment-softmax-aggregates via indicator matmuls into PSUM.
  - Softmax skips the segment-max subtraction: logits are O(30) so exp stays
    comfortably inside fp32 range, and out = (sum p*xl)/(sum p) is exact.
  - Edges are host-packed into chunks of 128; each group of 128 dst nodes
    owns CG chunks (padded with dummy edges, dstg=200 -> zero indicator row).
  - All per-core inputs ship as ONE packed [128, TOT] f32 tensor (int32
    index columns bitcast) so every consumer waits on a single DMA lane
    (walrus allows only one sync wait on a Matmult).

kernel(**inputs) -> [100000, 64] fp32.
"""
import os
import numpy as np
import jax
from jax.sharding import Mesh, PartitionSpec, NamedSharding
from jax.experimental.shard_map import shard_map

import concourse.bacc as bacc
import concourse.tile as tile
from concourse import mybir, bass
from concourse.bass2jax import (_bass_exec_p, install_neuronx_cc_hook,
                                partition_id_tensor)

F32 = mybir.dt.float32
BF16 = mybir.dt.bfloat16
I32 = mybir.dt.int32
AF = mybir.ActivationFunctionType
ALU = mybir.AluOpType

N = 100000
IN = 128
HC = 64
NCORES = 8
PER = N // NCORES            # 12500
PERPAD = 12544               # 98 * 128
G = PERPAD // 128            # 98 groups/core
NEG = 0.2

# packed-input column offsets (CG-independent part)
OF_XT = 0
OF_CST = 12544
OF_W1 = OF_CST + 896
OF_W2 = OF_W1 + 128
OF_IDX = OF_W2 + 128          # 13696; then srcg|dstg|wcol each NCH wide
# cst sub-offsets (relative to OF_CST)
B1, B2 = 0, 128
ATT1, ATT2 = 256, 320
BIA1, BIA2 = 384, 448
WE1, WE2 = 512, 576
IOTA, IDEN = 640, 768

_cache = {}
_PHASE = int(os.environ.get("K2_PHASE", "4"))


# ----------------------------------------------------------------- builder
def _build(CG, phase=4, abl=frozenset()):
    NCH = G * CG
    TOT = OF_IDX + 3 * NCH
    nc = bacc.Bacc("TRN2", target_bir_lowering=False, debug=False)
    t_mega = nc.dram_tensor("mega", [128, TOT], F32, kind="ExternalInput")
    t_out = nc.dram_tensor("out", [PERPAD, 64], F32, kind="ExternalOutput")

    with tile.TileContext(nc) as tc:
        with tc.tile_pool(name="dram", bufs=1, space="DRAM") as dpool, \
             tc.tile_pool(name="big", bufs=1) as bigp, \
             tc.tile_pool(name="sb", bufs=2) as pool, \
             tc.tile_pool(name="gat", bufs=2) as gpool, \
             tc.tile_pool(name="pd", bufs=2, space="PSUM") as psd, \
             tc.tile_pool(name="pt", bufs=2, space="PSUM") as pst, \
             tc.tile_pool(name="pz", bufs=1, space="PSUM") as psz, \
             tc.tile_pool(name="pu", bufs=2, space="PSUM") as psu:

            xl1_sh = dpool.tile([PERPAD, 64], F32)
            xr1_tab = dpool.tile([PERPAD, 64], F32)
            xl1_tab = dpool.tile([NCORES * PERPAD, 64], F32, addr_space="Shared")
            xl2_sh = dpool.tile([PERPAD, 64], F32)
            xr2_tab = dpool.tile([PERPAD, 64], F32)
            xl2_tab = dpool.tile([NCORES * PERPAD, 64], F32, addr_space="Shared")

            mega = bigp.tile([128, TOT], F32)
            nc.sync.dma_start(out=mega[:], in_=t_mega[:])
            xT = mega[:, OF_XT:OF_XT + PERPAD]
            cst = mega[:, OF_CST:OF_CST + 896]
            W1 = mega[:, OF_W1:OF_W1 + 128]
            W2 = mega[:, OF_W2:OF_W2 + 128]
            srcg = mega[:, OF_IDX + 0 * NCH:OF_IDX + 1 * NCH].bitcast(I32)
            dstg = mega[:, OF_IDX + 1 * NCH:OF_IDX + 2 * NCH]
            wcol = mega[:, OF_IDX + 2 * NCH:OF_IDX + 3 * NCH]
            hT = bigp.tile([128, G * 64], F32)   # tile t -> part 64*(t%2), col (t//2)*128

            # ---------------- dense 1
            for t in range(G):
                pd = psd.tile([128, 128], F32, space="PSUM", tag="pd")
                nc.tensor.matmul(pd[:], lhsT=xT[:, t * 128:(t + 1) * 128],
                                 rhs=W1[:], start=True, stop=True)
                xlr = pool.tile([128, 128], F32, tag="xlr")
                nc.vector.tensor_add(xlr[:], pd[:], cst[:, B1:B1 + 128])
                nc.sync.dma_start(out=xl1_sh[t * 128:(t + 1) * 128, :], in_=xlr[:, 0:64])
                nc.sync.dma_start(out=xr1_tab[t * 128:(t + 1) * 128, :], in_=xlr[:, 64:128])

            nc.gpsimd.collective_compute(
                "AllGather", ALU.bypass,
                replica_groups=[list(range(NCORES))],
                ins=[xl1_sh[:]], outs=[xl1_tab[:]])

            if phase == 1:
                for t in range(G):
                    ot = pool.tile([128, 64], F32, tag="otp1")
                    nc.sync.dma_start(out=ot[:], in_=xl1_tab[t * 128:(t + 1) * 128, :])
                    nc.sync.dma_start(out=t_out[t * 128:(t + 1) * 128, :], in_=ot[:])

            # ---------------- edge phase 1 (heads=2, c=32)
            CGW = CG * 64
            for g in range(G if phase >= 2 else 0):
                pu = psu.tile([128, 66], F32, space="PSUM", tag="pu")
                xl_G = gpool.tile([128, CGW], F32, tag="xl")
                for j in range(CG):
                    c = g * CG + j
                    nc.gpsimd.indirect_dma_start(
                        out=xl_G[:, j * 64:(j + 1) * 64], out_offset=None,
                        in_=xl1_tab[:],
                        in_offset=bass.IndirectOffsetOnAxis(ap=srcg[:, c:c + 1], axis=0))
                xr_grp = gpool.tile([128, 64], F32, tag="xr")
                nc.sync.dma_start(out=xr_grp[:], in_=xr1_tab[g * 128:(g + 1) * 128, :])
                # z0 = xl + We1*w  (group-wide)
                t1G = pool.tile([128, CGW], F32, tag="t1")
                nc.vector.tensor_tensor(
                    out=t1G.rearrange("p (j c) -> p j c", c=64),
                    in0=cst[:, WE1:WE1 + 64].rearrange("p (o c) -> p o c", o=1)
                        .to_broadcast([128, CG, 64]),
                    in1=wcol[:, g * CG:(g + 1) * CG].to_broadcast([128, CG, 64]),
                    op=ALU.mult)
                z0G = pool.tile([128, CGW], F32, tag="z0")
                nc.vector.tensor_add(z0G[:], xl_G[:], t1G[:])
                indG = pool.tile([128, CG * 128], F32, tag="ind")
                for j in range(CG):
                    c = g * CG + j
                    nc.vector.tensor_tensor(
                        out=indG[:, j * 128:(j + 1) * 128],
                        in0=dstg[:, c:c + 1].to_broadcast([128, 128]),
                        in1=cst[:, IOTA:IOTA + 128], op=ALU.is_equal)
                pzG = psz.tile([128, CGW], F32, space="PSUM", tag="pz")
                for j in range(CG):
                    ptt = pst.tile([128, 128], F32, space="PSUM", tag="ptt")
                    nc.tensor.transpose(out=ptt[:], in_=indG[:, j * 128:(j + 1) * 128],
                                        identity=cst[:, IDEN:IDEN + 128])
                    indT = pool.tile([128, 128], F32, tag="indT", bufs=3)
                    nc.vector.tensor_copy(indT[:], ptt[:])
                    nc.tensor.matmul(pzG[:, j * 64:(j + 1) * 64], lhsT=indT[:],
                                     rhs=xr_grp[:], start=True, stop=True)
                zG = pool.tile([128, CGW], F32, tag="z")
                nc.vector.tensor_add(zG[:], z0G[:], pzG[:])
                lrG = pool.tile([128, CGW], F32, tag="lr")
                nc.scalar.activation(lrG[:], zG[:], AF.Prelu, alpha=NEG)
                lrwG = pool.tile([128, CGW], F32, tag="lrw")
                nc.vector.tensor_tensor(
                    out=lrwG.rearrange("p (j c) -> p j c", c=64),
                    in0=lrG.rearrange("p (j c) -> p j c", c=64),
                    in1=cst[:, ATT1:ATT1 + 64].rearrange("p (o c) -> p o c", o=1)
                        .to_broadcast([128, CG, 64]),
                    op=ALU.mult)
                laG = pool.tile([128, CG * 2], F32, tag="la")
                nc.vector.tensor_reduce(
                    out=laG[:], in_=lrwG.rearrange("p (a c) -> p a c", c=32),
                    axis=mybir.AxisListType.X, op=ALU.add)
                vtG = pool.tile([128, CG * 66], F32, tag="vt")
                vt3 = vtG.rearrange("p (j k) -> p j k", k=66)
                nc.scalar.activation(vt3[:, :, 64:66],
                                     laG.rearrange("p (j h) -> p j h", h=2), AF.Exp)
                xl3 = xl_G.rearrange("p (j c) -> p j c", c=64)
                for h in range(2):
                    nc.vector.tensor_tensor(
                        out=vt3[:, :, h * 32:(h + 1) * 32],
                        in0=xl3[:, :, h * 32:(h + 1) * 32],
                        in1=vt3[:, :, 64 + h:65 + h].to_broadcast([128, CG, 32]),
                        op=ALU.mult)
                for j in range(CG):
                    nc.tensor.matmul(pu[:], lhsT=indG[:, j * 128:(j + 1) * 128],
                                     rhs=vtG[:, j * 66:(j + 1) * 66],
                                     start=(j == 0), stop=(j == CG - 1))
                # finalize group: h = relu(u/s + bias1), store transposed
                # (clamp s away from 0 so empty padding rows give 0, not NaN)
                sm = pool.tile([128, 2], F32, tag="sm")
                nc.vector.tensor_scalar_max(sm[:], pu[:, 64:66], 1e-30)
                rec = pool.tile([128, 2], F32, tag="rec")
                nc.vector.reciprocal(rec[:], sm[:])
                h = pool.tile([128, 64], F32, tag="h")
                nc.vector.tensor_mul(h[:, 0:32], pu[:, 0:32],
                                     rec[:, 0:1].to_broadcast([128, 32]))
                nc.vector.tensor_mul(h[:, 32:64], pu[:, 32:64],
                                     rec[:, 1:2].to_broadcast([128, 32]))
                nc.vector.tensor_add(h[:], h[:], cst[:, BIA1:BIA1 + 64])
                hr = pool.tile([128, 64], F32, tag="hr")
                nc.vector.tensor_scalar_max(hr[:], h[:], 0.0)
                ptt = pst.tile([64, 128], F32, space="PSUM", tag="ptt")
                nc.tensor.transpose(out=ptt[:], in_=hr[:], identity=cst[:, IDEN:IDEN + 128])
                po = 64 * (g % 2)
                nc.vector.tensor_copy(hT[po:po + 64, (g // 2) * 128:(g // 2) * 128 + 128],
                                      ptt[:])
                if phase == 2:
                    nc.sync.dma_start(out=t_out[g * 128:(g + 1) * 128, :], in_=hr[:])

            # ---------------- dense 2
            for t in range(G if phase >= 3 else 0):
                po = 64 * (t % 2)
                pd = psd.tile([128, 128], F32, space="PSUM", tag="pd")
                nc.tensor.matmul(pd[:], lhsT=hT[po:po + 64, (t // 2) * 128:(t // 2) * 128 + 128],
                                 rhs=W2[po:po + 64, :], start=True, stop=True)
                xlr = pool.tile([128, 128], F32, tag="xlr")
                nc.vector.tensor_add(xlr[:], pd[:], cst[:, B2:B2 + 128])
                nc.sync.dma_start(out=xl2_sh[t * 128:(t + 1) * 128, :], in_=xlr[:, 0:64])
                nc.sync.dma_start(out=xr2_tab[t * 128:(t + 1) * 128, :], in_=xlr[:, 64:128])

            if phase == 31:
                for t in range(G):
                    ot = pool.tile([128, 64], F32, tag="otp3")
                    nc.sync.dma_start(out=ot[:], in_=xl2_sh[t * 128:(t + 1) * 128, :])
                    nc.sync.dma_start(out=t_out[t * 128:(t + 1) * 128, :], in_=ot[:])

            if phase >= 3 and phase != 31:
                nc.gpsimd.collective_compute(
                    "AllGather", ALU.bypass,
                    replica_groups=[list(range(NCORES))],
                    ins=[xl2_sh[:]], outs=[xl2_tab[:]])

            if phase == 3:
                for t in range(G):
                    ot = pool.tile([128, 64], F32, tag="otp3")
                    nc.sync.dma_start(out=ot[:], in_=xl2_tab[t * 128:(t + 1) * 128, :])
                    nc.sync.dma_start(out=t_out[t * 128:(t + 1) * 128, :], in_=ot[:])

            # ---------------- edge phase 2 (heads=1, c=64)
            for g in range(G if phase >= 4 and phase != 31 else 0):
                pu = psu.tile([128, 65], F32, space="PSUM", tag="pu")
                xl_G = gpool.tile([128, CGW], F32, tag="xl2")
                for j in range(CG):
                    c = g * CG + j
                    nc.gpsimd.indirect_dma_start(
                        out=xl_G[:, j * 64:(j + 1) * 64], out_offset=None,
                        in_=xl2_tab[:],
                        in_offset=bass.IndirectOffsetOnAxis(ap=srcg[:, c:c + 1], axis=0))
                xr_grp = gpool.tile([128, 64], F32, tag="xr2")
                nc.sync.dma_start(out=xr_grp[:], in_=xr2_tab[g * 128:(g + 1) * 128, :])
                t1G = pool.tile([128, CGW], F32, tag="t12")
                nc.vector.tensor_tensor(
                    out=t1G.rearrange("p (j c) -> p j c", c=64),
                    in0=cst[:, WE2:WE2 + 64].rearrange("p (o c) -> p o c", o=1)
                        .to_broadcast([128, CG, 64]),
                    in1=wcol[:, g * CG:(g + 1) * CG].to_broadcast([128, CG, 64]),
                    op=ALU.mult)
                z0G = pool.tile([128, CGW], F32, tag="z02")
                nc.vector.tensor_add(z0G[:], xl_G[:], t1G[:])
                indG = pool.tile([128, CG * 128], F32, tag="ind2")
                for j in range(CG):
                    c = g * CG + j
                    nc.vector.tensor_tensor(
                        out=indG[:, j * 128:(j + 1) * 128],
                        in0=dstg[:, c:c + 1].to_broadcast([128, 128]),
                        in1=cst[:, IOTA:IOTA + 128], op=ALU.is_equal)
                pzG = psz.tile([128, CGW], F32, space="PSUM", tag="pz")
                for j in range(CG):
                    ptt = pst.tile([128, 128], F32, space="PSUM", tag="ptt")
                    nc.tensor.transpose(out=ptt[:], in_=indG[:, j * 128:(j + 1) * 128],
                                        identity=cst[:, IDEN:IDEN + 128])
                    indT = pool.tile([128, 128], F32, tag="indT", bufs=3)
                    nc.vector.tensor_copy(indT[:], ptt[:])
                    nc.tensor.matmul(pzG[:, j * 64:(j + 1) * 64], lhsT=indT[:],
                                     rhs=xr_grp[:], start=True, stop=True)
                zG = pool.tile([128, CGW], F32, tag="z2")
                nc.vector.tensor_add(zG[:], z0G[:], pzG[:])
                lrG = pool.tile([128, CGW], F32, tag="lr2")
                nc.scalar.activation(lrG[:], zG[:], AF.Prelu, alpha=NEG)
                lrwG = pool.tile([128, CGW], F32, tag="lrw2")
                nc.vector.tensor_tensor(
                    out=lrwG.rearrange("p (j c) -> p j c", c=64),
                    in0=lrG.rearrange("p (j c) -> p j c", c=64),
                    in1=cst[:, ATT2:ATT2 + 64].rearrange("p (o c) -> p o c", o=1)
                        .to_broadcast([128, CG, 64]),
                    op=ALU.mult)
                laG = pool.tile([128, CG], F32, tag="la2")
                nc.vector.tensor_reduce(
                    out=laG[:], in_=lrwG.rearrange("p (a c) -> p a c", c=64),
                    axis=mybir.AxisListType.X, op=ALU.add)
                vtG = pool.tile([128, CG * 65], F32, tag="vt2")
                vt3 = vtG.rearrange("p (j k) -> p j k", k=65)
                nc.scalar.activation(vt3[:, :, 64:65],
                                     laG.rearrange("p (j h) -> p j h", h=1), AF.Exp)
                xl3 = xl_G.rearrange("p (j c) -> p j c", c=64)
                nc.vector.tensor_tensor(
                    out=vt3[:, :, 0:64], in0=xl3[:, :, :],
                    in1=vt3[:, :, 64:65].to_broadcast([128, CG, 64]),
                    op=ALU.mult)
                for j in range(CG):
                    nc.tensor.matmul(pu[:], lhsT=indG[:, j * 128:(j + 1) * 128],
                                     rhs=vtG[:, j * 65:(j + 1) * 65],
                                     start=(j == 0), stop=(j == CG - 1))
                sm = pool.tile([128, 1], F32, tag="sm2")
                nc.vector.tensor_scalar_max(sm[:], pu[:, 64:65], 1e-30)
                rec = pool.tile([128, 1], F32, tag="rec2")
                nc.vector.reciprocal(rec[:], sm[:])
                o = pool.tile([128, 64], F32, tag="o")
                nc.vector.tensor_mul(o[:], pu[:, 0:64], rec[:].to_broadcast([128, 64]))
                nc.vector.tensor_add(o[:], o[:], cst[:, BIA2:BIA2 + 64])
                nc.sync.dma_start(out=t_out[g * 128:(g + 1) * 128, :], in_=o[:])
    nc.compile()
    return nc


# ----------------------------------------------------------------- runner
def _make_runner(nc):
    install_neuronx_cc_hook()
    in_names, out_names, out_avals = [], [], []
    partition_name = nc.partition_id_tensor.name if nc.partition_id_tensor else None
    for alloc in nc.m.functions[0].allocations:
        if not isinstance(alloc, mybir.MemoryLocationSet):
            continue
        name = alloc.memorylocations[0].name
        if alloc.kind == "ExternalInput":
            if name != partition_name:
                in_names.append(name)
        elif alloc.kind == "ExternalOutput":
            out_names.append(name)
            out_avals.append(jax.core.ShapedArray(tuple(alloc.tensor_shape),
                                                  mybir.dt.np(alloc.dtype)))
    n_params = len(in_names)
    n_outs = len(out_avals)
    all_in_names = list(in_names) + list(out_names)
    if partition_name is not None:
        all_in_names.append(partition_name)

    def _body(*args):
        operands = list(args)
        if partition_name is not None:
            operands.append(partition_id_tensor())
        outs = _bass_exec_p.bind(
            *operands,
            out_avals=tuple(out_avals),
            in_names=tuple(all_in_names),
            out_names=tuple(out_names),
            lowering_input_output_aliases=(),
            sim_require_finite=True,
            sim_require_nnan=True,
            nc=nc,
        )
        return tuple(outs)

    devices = jax.devices()[:NCORES]
    mesh = Mesh(np.asarray(devices), ("core",))
    in_specs = (PartitionSpec("core"),) * (n_params + n_outs)
    out_specs = (PartitionSpec("core"),) * n_outs
    fn = jax.jit(shard_map(_body, mesh=mesh, in_specs=in_specs,
                           out_specs=out_specs, check_rep=False),
                 keep_unused=True)
    return fn, in_names, out_names, out_avals, mesh, devices


def _stage(runner, in_maps):
    """Device-put per-core inputs (plus zero output feeds) as sharded arrays."""
    fn, in_names, out_names, out_avals, mesh, devices = runner
    staged = []
    for name in in_names:
        shards = [jax.device_put(np.ascontiguousarray(in_maps[k][name]), devices[k])
                  for k in range(NCORES)]
        jax.block_until_ready(shards)
        shp = in_maps[0][name].shape
        arr = jax.make_array_from_single_device_arrays(
            (NCORES * shp[0],) + tuple(shp[1:]),
            NamedSharding(mesh, PartitionSpec("core")), shards)
        staged.append(arr)
    for av in out_avals:
        z = np.zeros(av.shape, av.dtype)
        shards = [jax.device_put(z, d) for d in devices]
        jax.block_until_ready(shards)
        arr = jax.make_array_from_single_device_arrays(
            (NCORES * av.shape[0],) + tuple(av.shape[1:]),
            NamedSharding(mesh, PartitionSpec("core")), shards)
        staged.append(arr)
    return staged


# ----------------------------------------------------------------- host prep
def _host_prep(x, edge_index, edge_weight,
               W1l, b1l, W1r, b1r, We1, att1, bias1,
               W2l, b2l, W2r, b2r, We2, att2, bias2):
    src = edge_index[0].astype(np.int64)
    dst = edge_index[1].astype(np.int64)
    ew = edge_weight[:, 0].astype(np.float64)
    deg = np.bincount(dst, minlength=N).astype(np.float64)
    wsum = np.bincount(dst, weights=ew, minlength=N)
    loop_w = (wsum / np.maximum(deg, 1.0)).astype(np.float32)

    allsrc = np.concatenate([src, np.arange(N, dtype=np.int64)])
    alldst = np.concatenate([dst, np.arange(N, dtype=np.int64)])
    allw = np.concatenate([edge_weight[:, 0].astype(np.float32), loop_w])
    order = np.argsort(alldst, kind="stable")
    ss, ds, ws = allsrc[order], alldst[order], allw[order]

    core = ds // PER
    loc = ds % PER
    gid = core * G + loc // 128
    gcnt = np.bincount(gid, minlength=NCORES * G)
    CG = max(2, int(np.ceil(gcnt.max() / 128.0)))
    NCH = G * CG
    gstart = np.zeros(NCORES * G + 1, np.int64)
    np.cumsum(gcnt, out=gstart[1:])
    slot = np.arange(ds.shape[0], dtype=np.int64) - gstart[gid]

    SLOTS = NCH * 128
    srcg = np.zeros((NCORES, SLOTS), np.int32)
    dstl = np.zeros((NCORES, SLOTS), np.int32)
    dstg = np.full((NCORES, SLOTS), 200.0, np.float32)
    wpad = np.zeros((NCORES, SLOTS), np.float32)
    ce = gid // G
    pos = (gid % G) * (CG * 128) + slot
    srcg[ce, pos] = ((ss // PER) * PERPAD + ss % PER).astype(np.int32)
    dstl[ce, pos] = loc.astype(np.int32)
    dstg[ce, pos] = (loc % 128).astype(np.float32)
    wpad[ce, pos] = ws

    def colmaj(a):  # [SLOTS] -> [128, NCH]
        return np.ascontiguousarray(a.reshape(NCH, 128).T)

    cst = np.zeros((128, 896), np.float32)
    cst[:, 0:128] = np.concatenate([b1l, b1r])[None, :]
    cst[:, 128:256] = np.concatenate([b2l, b2r])[None, :]
    cst[:, 256:320] = att1.reshape(-1)[None, :]
    cst[:, 320:384] = att2.reshape(-1)[None, :]
    cst[:, 384:448] = bias1[None, :]
    cst[:, 448:512] = bias2[None, :]
    cst[:, 512:576] = We1.reshape(-1)[None, :]
    cst[:, 576:640] = We2.reshape(-1)[None, :]
    cst[:, 640:768] = np.arange(128, dtype=np.float32)[None, :]
    cst[:, 768:896] = np.eye(128, dtype=np.float32)

    W1lr = np.concatenate([W1l, W1r], axis=1)           # [128,128]
    W2cat = np.concatenate([W2l, W2r], axis=1)          # [64,128]
    W2lr = np.concatenate([W2cat, W2cat], axis=0)       # [128,128]

    TOT = OF_IDX + 3 * NCH
    in_maps = []
    for k in range(NCORES):
        mega = np.zeros((128, TOT), np.float32)
        mega[:, OF_XT + 0:OF_XT + PER] = x[k * PER:(k + 1) * PER].T
        mega[:, OF_CST:OF_CST + 896] = cst
        mega[:, OF_W1:OF_W1 + 128] = W1lr
        mega[:, OF_W2:OF_W2 + 128] = W2lr
        mega[:, OF_IDX + 0 * NCH:OF_IDX + 1 * NCH] = colmaj(srcg[k]).view(np.float32)
        mega[:, OF_IDX + 1 * NCH:OF_IDX + 2 * NCH] = colmaj(dstg[k])
        mega[:, OF_IDX + 2 * NCH:OF_IDX + 3 * NCH] = colmaj(wpad[k])
        in_maps.append(dict(mega=mega))
    return CG, in_maps


def _get_program(CG):
    key = ("fused", CG, _PHASE)
    if key not in _cache:
        nc = _build(CG, _PHASE)
        _cache[key] = (nc, _make_runner(nc))
    return _cache[key]


def _execute(runner, staged):
    fn = runner[0]
    out = fn(*staged)
    return out


def kernel(x, edge_index, edge_weight,
           W1l, b1l, W1r, b1r, We1, att1, bias1,
           W2l, b2l, W2r, b2r, We2, att2, bias2):
    f32 = lambda a: np.asarray(a, np.float32)
    CG, in_maps = _host_prep(
        f32(x), np.asarray(edge_index), f32(edge_weight),
        f32(W1l), f32(b1l), f32(W1r), f32(b1r), f32(We1), f32(att1), f32(bias1),
        f32(W2l), f32(b2l), f32(W2r), f32(b2r), f32(We2), f32(att2), f32(bias2))
    nc, runner = _get_program(CG)
    staged = _stage(runner, in_maps)
    out = _execute(runner, staged)
    glob = np.asarray(out[0])          # [8*PERPAD, 64]
    res = np.empty((N, 64), np.float32)
    for k in range(NCORES):
        res[k * PER:(k + 1) * PER] = glob[k * PERPAD:k * PERPAD + PER]
    return res



# revision 24
# speedup vs baseline: 1.3491x; 1.2481x over previous
"""GATv2 (2-layer) fully fused on 8 Trainium2 NeuronCores.

Design (dst-range edge sharding):
  - Nodes sharded 12500/core (padded 12544). Edges (incl. mean-fill self
    loops) sorted by dst and assigned to the core owning dst.
  - Per core: dense transforms xl/xr = x @ Wl|Wr + b on PE; AllGather of the
    per-core xl shards builds a full local xl table in each core's HBM;
    edge phase gathers xl[src] (indirect DMA from the gathered table) and
    xr[dst] (indirect DMA from the local xr table), computes GATv2 scores,
    and segment-softmax-aggregates via indicator matmuls into PSUM.
  - Softmax skips the segment-max subtraction: logits are O(30) so exp stays
    comfortably inside fp32 range, and out = (sum p*xl)/(sum p) is exact.
  - Edges are host-packed into chunks of 128; each group of 128 dst nodes
    owns CG chunks (padded with dummy edges, dstg=200 -> zero indicator row).
  - All per-core inputs ship as ONE packed [128, TOT] f32 tensor (int32
    index columns bitcast) so every consumer waits on a single DMA lane
    (walrus allows only one sync wait on a Matmult).

kernel(**inputs) -> [100000, 64] fp32.
"""
import os
import numpy as np
import jax
from jax.sharding import Mesh, PartitionSpec, NamedSharding
from jax.experimental.shard_map import shard_map

import concourse.bacc as bacc
import concourse.tile as tile
from concourse import mybir, bass
from concourse.bass2jax import (_bass_exec_p, install_neuronx_cc_hook,
                                partition_id_tensor)

F32 = mybir.dt.float32
BF16 = mybir.dt.bfloat16
I32 = mybir.dt.int32
AF = mybir.ActivationFunctionType
ALU = mybir.AluOpType

N = 100000
IN = 128
HC = 64
NCORES = 8
PER = N // NCORES            # 12500
PERPAD = 12544               # 98 * 128
G = PERPAD // 128            # 98 groups/core
NEG = 0.2

# packed-input column offsets (CG-independent part)
OF_XT = 0
OF_CST = 12544
OF_W1 = OF_CST + 896
OF_W2 = OF_W1 + 128
OF_IDX = OF_W2 + 128          # 13696; then srcg|dstg|wcol each NCH wide
# cst sub-offsets (relative to OF_CST)
B1, B2 = 0, 128
ATT1, ATT2 = 256, 320
BIA1, BIA2 = 384, 448
WE1, WE2 = 512, 576
IOTA, IDEN = 640, 768

_cache = {}
_PHASE = int(os.environ.get("K2_PHASE", "4"))


# ----------------------------------------------------------------- builder
def _build(CG, phase=4, abl=frozenset()):
    NCH = G * CG
    TOT = OF_IDX + 3 * NCH
    nc = bacc.Bacc("TRN2", target_bir_lowering=False, debug=False)
    t_mega = nc.dram_tensor("mega", [128, TOT], F32, kind="ExternalInput")
    t_out = nc.dram_tensor("out", [PERPAD, 64], F32, kind="ExternalOutput")

    with tile.TileContext(nc) as tc:
        with tc.tile_pool(name="dram", bufs=1, space="DRAM") as dpool, \
             tc.tile_pool(name="big", bufs=1) as bigp, \
             tc.tile_pool(name="sb", bufs=2) as pool, \
             tc.tile_pool(name="gat", bufs=2) as gpool, \
             tc.tile_pool(name="pd", bufs=2, space="PSUM") as psd, \
             tc.tile_pool(name="pt", bufs=2, space="PSUM") as pst, \
             tc.tile_pool(name="pz", bufs=1, space="PSUM") as psz, \
             tc.tile_pool(name="pu", bufs=2, space="PSUM") as psu:

            xl1_sh = dpool.tile([PERPAD, 64], BF16)
            xr1_tab = dpool.tile([PERPAD, 64], F32)
            xl1_tab = dpool.tile([NCORES * PERPAD, 64], BF16, addr_space="Shared")
            xl2_sh = dpool.tile([PERPAD, 64], BF16)
            xr2_tab = dpool.tile([PERPAD, 64], F32)
            xl2_tab = dpool.tile([NCORES * PERPAD, 64], BF16, addr_space="Shared")

            mega = bigp.tile([128, TOT], F32)
            nc.sync.dma_start(out=mega[:], in_=t_mega[:])
            xT = mega[:, OF_XT:OF_XT + PERPAD]
            cst = mega[:, OF_CST:OF_CST + 896]
            W1 = mega[:, OF_W1:OF_W1 + 128]
            W2 = mega[:, OF_W2:OF_W2 + 128]
            srcg = mega[:, OF_IDX + 0 * NCH:OF_IDX + 1 * NCH].bitcast(I32)
            dstg = mega[:, OF_IDX + 1 * NCH:OF_IDX + 2 * NCH]
            wcol = mega[:, OF_IDX + 2 * NCH:OF_IDX + 3 * NCH]
            hT = bigp.tile([128, G * 64], F32)   # tile t -> part 64*(t%2), col (t//2)*128

            # ---------------- dense 1
            for t in range(G):
                pd = psd.tile([128, 128], F32, space="PSUM", tag="pd")
                nc.tensor.matmul(pd[:], lhsT=xT[:, t * 128:(t + 1) * 128],
                                 rhs=W1[:], start=True, stop=True)
                xlb = pool.tile([128, 64], BF16, tag="xlb")
                nc.vector.tensor_add(xlb[:], pd[:, 0:64], cst[:, B1:B1 + 64])
                xrf = pool.tile([128, 64], F32, tag="xrf")
                nc.vector.tensor_add(xrf[:], pd[:, 64:128], cst[:, B1 + 64:B1 + 128])
                nc.sync.dma_start(out=xl1_sh[t * 128:(t + 1) * 128, :], in_=xlb[:])
                nc.sync.dma_start(out=xr1_tab[t * 128:(t + 1) * 128, :], in_=xrf[:])

            nc.gpsimd.collective_compute(
                "AllGather", ALU.bypass,
                replica_groups=[list(range(NCORES))],
                ins=[xl1_sh[:]], outs=[xl1_tab[:]])

            if phase == 1:
                for t in range(G):
                    ot = pool.tile([128, 64], F32, tag="otp1")
                    nc.sync.dma_start(out=ot[:], in_=xl1_tab[t * 128:(t + 1) * 128, :])
                    nc.sync.dma_start(out=t_out[t * 128:(t + 1) * 128, :], in_=ot[:])

            # ---------------- edge phase 1 (heads=2, c=32)
            CGW = CG * 64
            for g in range(G if phase >= 2 else 0):
                pu = psu.tile([128, 66], F32, space="PSUM", tag="pu")
                xl_G = gpool.tile([128, CGW], BF16, tag="xl", bufs=3)
                for j in range(CG):
                    c = g * CG + j
                    nc.gpsimd.indirect_dma_start(
                        out=xl_G[:, j * 64:(j + 1) * 64], out_offset=None,
                        in_=xl1_tab[:],
                        in_offset=bass.IndirectOffsetOnAxis(ap=srcg[:, c:c + 1], axis=0))
                xr_grp = gpool.tile([128, 64], F32, tag="xr")
                nc.sync.dma_start(out=xr_grp[:], in_=xr1_tab[g * 128:(g + 1) * 128, :])
                # z0 = xl + We1*w  (group-wide)
                t1G = pool.tile([128, CGW], F32, tag="t1")
                nc.vector.tensor_tensor(
                    out=t1G.rearrange("p (j c) -> p j c", c=64),
                    in0=cst[:, WE1:WE1 + 64].rearrange("p (o c) -> p o c", o=1)
                        .to_broadcast([128, CG, 64]),
                    in1=wcol[:, g * CG:(g + 1) * CG].to_broadcast([128, CG, 64]),
                    op=ALU.mult)
                z0G = pool.tile([128, CGW], F32, tag="z0")
                nc.vector.tensor_add(z0G[:], xl_G[:], t1G[:])
                indG = pool.tile([128, CG * 128], F32, tag="ind")
                for j in range(CG):
                    c = g * CG + j
                    nc.vector.tensor_tensor(
                        out=indG[:, j * 128:(j + 1) * 128],
                        in0=dstg[:, c:c + 1].to_broadcast([128, 128]),
                        in1=cst[:, IOTA:IOTA + 128], op=ALU.is_equal)
                pzG = psz.tile([128, CGW], F32, space="PSUM", tag="pz")
                for j in range(CG):
                    ptt = pst.tile([128, 128], F32, space="PSUM", tag="ptt")
                    nc.tensor.transpose(out=ptt[:], in_=indG[:, j * 128:(j + 1) * 128],
                                        identity=cst[:, IDEN:IDEN + 128])
                    indT = pool.tile([128, 128], F32, tag="indT", bufs=3)
                    nc.vector.tensor_copy(indT[:], ptt[:])
                    nc.tensor.matmul(pzG[:, j * 64:(j + 1) * 64], lhsT=indT[:],
                                     rhs=xr_grp[:], start=True, stop=True)
                zG = pool.tile([128, CGW], F32, tag="z")
                nc.vector.tensor_add(zG[:], z0G[:], pzG[:])
                lrG = pool.tile([128, CGW], F32, tag="lr")
                nc.scalar.activation(lrG[:], zG[:], AF.Prelu, alpha=NEG)
                lrwG = pool.tile([128, CGW], F32, tag="lrw")
                nc.vector.tensor_tensor(
                    out=lrwG.rearrange("p (j c) -> p j c", c=64),
                    in0=lrG.rearrange("p (j c) -> p j c", c=64),
                    in1=cst[:, ATT1:ATT1 + 64].rearrange("p (o c) -> p o c", o=1)
                        .to_broadcast([128, CG, 64]),
                    op=ALU.mult)
                laG = pool.tile([128, CG * 2], F32, tag="la")
                nc.vector.tensor_reduce(
                    out=laG[:], in_=lrwG.rearrange("p (a c) -> p a c", c=32),
                    axis=mybir.AxisListType.X, op=ALU.add)
                vtG = pool.tile([128, CG * 66], F32, tag="vt")
                vt3 = vtG.rearrange("p (j k) -> p j k", k=66)
                nc.scalar.activation(vt3[:, :, 64:66],
                                     laG.rearrange("p (j h) -> p j h", h=2), AF.Exp)
                xl3 = xl_G.rearrange("p (j c) -> p j c", c=64)
                for h in range(2):
                    nc.vector.tensor_tensor(
                        out=vt3[:, :, h * 32:(h + 1) * 32],
                        in0=xl3[:, :, h * 32:(h + 1) * 32],
                        in1=vt3[:, :, 64 + h:65 + h].to_broadcast([128, CG, 32]),
                        op=ALU.mult)
                for j in range(CG):
                    nc.tensor.matmul(pu[:], lhsT=indG[:, j * 128:(j + 1) * 128],
                                     rhs=vtG[:, j * 66:(j + 1) * 66],
                                     start=(j == 0), stop=(j == CG - 1))
                # finalize group: h = relu(u/s + bias1), store transposed
                # (clamp s away from 0 so empty padding rows give 0, not NaN)
                sm = pool.tile([128, 2], F32, tag="sm")
                nc.vector.tensor_scalar_max(sm[:], pu[:, 64:66], 1e-30)
                rec = pool.tile([128, 2], F32, tag="rec")
                nc.vector.reciprocal(rec[:], sm[:])
                h = pool.tile([128, 64], F32, tag="h")
                nc.vector.tensor_mul(h[:, 0:32], pu[:, 0:32],
                                     rec[:, 0:1].to_broadcast([128, 32]))
                nc.vector.tensor_mul(h[:, 32:64], pu[:, 32:64],
                                     rec[:, 1:2].to_broadcast([128, 32]))
                nc.vector.tensor_add(h[:], h[:], cst[:, BIA1:BIA1 + 64])
                hr = pool.tile([128, 64], F32, tag="hr")
                nc.vector.tensor_scalar_max(hr[:], h[:], 0.0)
                ptt = pst.tile([64, 128], F32, space="PSUM", tag="ptt")
                nc.tensor.transpose(out=ptt[:], in_=hr[:], identity=cst[:, IDEN:IDEN + 128])
                po = 64 * (g % 2)
                nc.vector.tensor_copy(hT[po:po + 64, (g // 2) * 128:(g // 2) * 128 + 128],
                                      ptt[:])
                if phase == 2:
                    nc.sync.dma_start(out=t_out[g * 128:(g + 1) * 128, :], in_=hr[:])

            # ---------------- dense 2
            for t in range(G if phase >= 3 else 0):
                po = 64 * (t % 2)
                pd = psd.tile([128, 128], F32, space="PSUM", tag="pd")
                nc.tensor.matmul(pd[:], lhsT=hT[po:po + 64, (t // 2) * 128:(t // 2) * 128 + 128],
                                 rhs=W2[po:po + 64, :], start=True, stop=True)
                xlb = pool.tile([128, 64], BF16, tag="xlb")
                nc.vector.tensor_add(xlb[:], pd[:, 0:64], cst[:, B2:B2 + 64])
                xrf = pool.tile([128, 64], F32, tag="xrf")
                nc.vector.tensor_add(xrf[:], pd[:, 64:128], cst[:, B2 + 64:B2 + 128])
                nc.sync.dma_start(out=xl2_sh[t * 128:(t + 1) * 128, :], in_=xlb[:])
                nc.sync.dma_start(out=xr2_tab[t * 128:(t + 1) * 128, :], in_=xrf[:])

            if phase == 31:
                for t in range(G):
                    ot = pool.tile([128, 64], F32, tag="otp3")
                    nc.sync.dma_start(out=ot[:], in_=xl2_sh[t * 128:(t + 1) * 128, :])
                    nc.sync.dma_start(out=t_out[t * 128:(t + 1) * 128, :], in_=ot[:])

            if phase >= 3 and phase != 31:
                nc.gpsimd.collective_compute(
                    "AllGather", ALU.bypass,
                    replica_groups=[list(range(NCORES))],
                    ins=[xl2_sh[:]], outs=[xl2_tab[:]])

            if phase == 3:
                for t in range(G):
                    ot = pool.tile([128, 64], F32, tag="otp3")
                    nc.sync.dma_start(out=ot[:], in_=xl2_tab[t * 128:(t + 1) * 128, :])
                    nc.sync.dma_start(out=t_out[t * 128:(t + 1) * 128, :], in_=ot[:])

            # ---------------- edge phase 2 (heads=1, c=64)
            for g in range(G if phase >= 4 and phase != 31 else 0):
                pu = psu.tile([128, 65], F32, space="PSUM", tag="pu")
                xl_G = gpool.tile([128, CGW], BF16, tag="xl2", bufs=3)
                for j in range(CG):
                    c = g * CG + j
                    nc.gpsimd.indirect_dma_start(
                        out=xl_G[:, j * 64:(j + 1) * 64], out_offset=None,
                        in_=xl2_tab[:],
                        in_offset=bass.IndirectOffsetOnAxis(ap=srcg[:, c:c + 1], axis=0))
                xr_grp = gpool.tile([128, 64], F32, tag="xr2")
                nc.sync.dma_start(out=xr_grp[:], in_=xr2_tab[g * 128:(g + 1) * 128, :])
                t1G = pool.tile([128, CGW], F32, tag="t12")
                nc.vector.tensor_tensor(
                    out=t1G.rearrange("p (j c) -> p j c", c=64),
                    in0=cst[:, WE2:WE2 + 64].rearrange("p (o c) -> p o c", o=1)
                        .to_broadcast([128, CG, 64]),
                    in1=wcol[:, g * CG:(g + 1) * CG].to_broadcast([128, CG, 64]),
                    op=ALU.mult)
                z0G = pool.tile([128, CGW], F32, tag="z02")
                nc.vector.tensor_add(z0G[:], xl_G[:], t1G[:])
                indG = pool.tile([128, CG * 128], F32, tag="ind2")
                for j in range(CG):
                    c = g * CG + j
                    nc.vector.tensor_tensor(
                        out=indG[:, j * 128:(j + 1) * 128],
                        in0=dstg[:, c:c + 1].to_broadcast([128, 128]),
                        in1=cst[:, IOTA:IOTA + 128], op=ALU.is_equal)
                pzG = psz.tile([128, CGW], F32, space="PSUM", tag="pz")
                for j in range(CG):
                    ptt = pst.tile([128, 128], F32, space="PSUM", tag="ptt")
                    nc.tensor.transpose(out=ptt[:], in_=indG[:, j * 128:(j + 1) * 128],
                                        identity=cst[:, IDEN:IDEN + 128])
                    indT = pool.tile([128, 128], F32, tag="indT", bufs=3)
                    nc.vector.tensor_copy(indT[:], ptt[:])
                    nc.tensor.matmul(pzG[:, j * 64:(j + 1) * 64], lhsT=indT[:],
                                     rhs=xr_grp[:], start=True, stop=True)
                zG = pool.tile([128, CGW], F32, tag="z2")
                nc.vector.tensor_add(zG[:], z0G[:], pzG[:])
                lrG = pool.tile([128, CGW], F32, tag="lr2")
                nc.scalar.activation(lrG[:], zG[:], AF.Prelu, alpha=NEG)
                lrwG = pool.tile([128, CGW], F32, tag="lrw2")
                nc.vector.tensor_tensor(
                    out=lrwG.rearrange("p (j c) -> p j c", c=64),
                    in0=lrG.rearrange("p (j c) -> p j c", c=64),
                    in1=cst[:, ATT2:ATT2 + 64].rearrange("p (o c) -> p o c", o=1)
                        .to_broadcast([128, CG, 64]),
                    op=ALU.mult)
                laG = pool.tile([128, CG], F32, tag="la2")
                nc.vector.tensor_reduce(
                    out=laG[:], in_=lrwG.rearrange("p (a c) -> p a c", c=64),
                    axis=mybir.AxisListType.X, op=ALU.add)
                vtG = pool.tile([128, CG * 65], F32, tag="vt2")
                vt3 = vtG.rearrange("p (j k) -> p j k", k=65)
                nc.scalar.activation(vt3[:, :, 64:65],
                                     laG.rearrange("p (j h) -> p j h", h=1), AF.Exp)
                xl3 = xl_G.rearrange("p (j c) -> p j c", c=64)
                nc.vector.tensor_tensor(
                    out=vt3[:, :, 0:64], in0=xl3[:, :, :],
                    in1=vt3[:, :, 64:65].to_broadcast([128, CG, 64]),
                    op=ALU.mult)
                for j in range(CG):
                    nc.tensor.matmul(pu[:], lhsT=indG[:, j * 128:(j + 1) * 128],
                                     rhs=vtG[:, j * 65:(j + 1) * 65],
                                     start=(j == 0), stop=(j == CG - 1))
                sm = pool.tile([128, 1], F32, tag="sm2")
                nc.vector.tensor_scalar_max(sm[:], pu[:, 64:65], 1e-30)
                rec = pool.tile([128, 1], F32, tag="rec2")
                nc.vector.reciprocal(rec[:], sm[:])
                o = pool.tile([128, 64], F32, tag="o")
                nc.vector.tensor_mul(o[:], pu[:, 0:64], rec[:].to_broadcast([128, 64]))
                nc.vector.tensor_add(o[:], o[:], cst[:, BIA2:BIA2 + 64])
                nc.sync.dma_start(out=t_out[g * 128:(g + 1) * 128, :], in_=o[:])
    nc.compile()
    return nc


# ----------------------------------------------------------------- runner
def _make_runner(nc):
    install_neuronx_cc_hook()
    in_names, out_names, out_avals = [], [], []
    partition_name = nc.partition_id_tensor.name if nc.partition_id_tensor else None
    for alloc in nc.m.functions[0].allocations:
        if not isinstance(alloc, mybir.MemoryLocationSet):
            continue
        name = alloc.memorylocations[0].name
        if alloc.kind == "ExternalInput":
            if name != partition_name:
                in_names.append(name)
        elif alloc.kind == "ExternalOutput":
            out_names.append(name)
            out_avals.append(jax.core.ShapedArray(tuple(alloc.tensor_shape),
                                                  mybir.dt.np(alloc.dtype)))
    n_params = len(in_names)
    n_outs = len(out_avals)
    all_in_names = list(in_names) + list(out_names)
    if partition_name is not None:
        all_in_names.append(partition_name)

    def _body(*args):
        operands = list(args)
        if partition_name is not None:
            operands.append(partition_id_tensor())
        outs = _bass_exec_p.bind(
            *operands,
            out_avals=tuple(out_avals),
            in_names=tuple(all_in_names),
            out_names=tuple(out_names),
            lowering_input_output_aliases=(),
            sim_require_finite=True,
            sim_require_nnan=True,
            nc=nc,
        )
        return tuple(outs)

    devices = jax.devices()[:NCORES]
    mesh = Mesh(np.asarray(devices), ("core",))
    in_specs = (PartitionSpec("core"),) * (n_params + n_outs)
    out_specs = (PartitionSpec("core"),) * n_outs
    fn = jax.jit(shard_map(_body, mesh=mesh, in_specs=in_specs,
                           out_specs=out_specs, check_rep=False),
                 keep_unused=True)
    return fn, in_names, out_names, out_avals, mesh, devices


def _stage(runner, in_maps):
    """Device-put per-core inputs (plus zero output feeds) as sharded arrays."""
    fn, in_names, out_names, out_avals, mesh, devices = runner
    staged = []
    for name in in_names:
        shards = [jax.device_put(np.ascontiguousarray(in_maps[k][name]), devices[k])
                  for k in range(NCORES)]
        jax.block_until_ready(shards)
        shp = in_maps[0][name].shape
        arr = jax.make_array_from_single_device_arrays(
            (NCORES * shp[0],) + tuple(shp[1:]),
            NamedSharding(mesh, PartitionSpec("core")), shards)
        staged.append(arr)
    for av in out_avals:
        z = np.zeros(av.shape, av.dtype)
        shards = [jax.device_put(z, d) for d in devices]
        jax.block_until_ready(shards)
        arr = jax.make_array_from_single_device_arrays(
            (NCORES * av.shape[0],) + tuple(av.shape[1:]),
            NamedSharding(mesh, PartitionSpec("core")), shards)
        staged.append(arr)
    return staged


# ----------------------------------------------------------------- host prep
def _host_prep(x, edge_index, edge_weight,
               W1l, b1l, W1r, b1r, We1, att1, bias1,
               W2l, b2l, W2r, b2r, We2, att2, bias2):
    src = edge_index[0].astype(np.int64)
    dst = edge_index[1].astype(np.int64)
    ew = edge_weight[:, 0].astype(np.float64)
    deg = np.bincount(dst, minlength=N).astype(np.float64)
    wsum = np.bincount(dst, weights=ew, minlength=N)
    loop_w = (wsum / np.maximum(deg, 1.0)).astype(np.float32)

    allsrc = np.concatenate([src, np.arange(N, dtype=np.int64)])
    alldst = np.concatenate([dst, np.arange(N, dtype=np.int64)])
    allw = np.concatenate([edge_weight[:, 0].astype(np.float32), loop_w])
    order = np.argsort(alldst, kind="stable")
    ss, ds, ws = allsrc[order], alldst[order], allw[order]

    core = ds // PER
    loc = ds % PER
    gid = core * G + loc // 128
    gcnt = np.bincount(gid, minlength=NCORES * G)
    CG = max(2, int(np.ceil(gcnt.max() / 128.0)))
    NCH = G * CG
    gstart = np.zeros(NCORES * G + 1, np.int64)
    np.cumsum(gcnt, out=gstart[1:])
    slot = np.arange(ds.shape[0], dtype=np.int64) - gstart[gid]

    SLOTS = NCH * 128
    srcg = np.zeros((NCORES, SLOTS), np.int32)
    dstl = np.zeros((NCORES, SLOTS), np.int32)
    dstg = np.full((NCORES, SLOTS), 200.0, np.float32)
    wpad = np.zeros((NCORES, SLOTS), np.float32)
    ce = gid // G
    pos = (gid % G) * (CG * 128) + slot
    srcg[ce, pos] = ((ss // PER) * PERPAD + ss % PER).astype(np.int32)
    dstl[ce, pos] = loc.astype(np.int32)
    dstg[ce, pos] = (loc % 128).astype(np.float32)
    wpad[ce, pos] = ws

    def colmaj(a):  # [SLOTS] -> [128, NCH]
        return np.ascontiguousarray(a.reshape(NCH, 128).T)

    cst = np.zeros((128, 896), np.float32)
    cst[:, 0:128] = np.concatenate([b1l, b1r])[None, :]
    cst[:, 128:256] = np.concatenate([b2l, b2r])[None, :]
    cst[:, 256:320] = att1.reshape(-1)[None, :]
    cst[:, 320:384] = att2.reshape(-1)[None, :]
    cst[:, 384:448] = bias1[None, :]
    cst[:, 448:512] = bias2[None, :]
    cst[:, 512:576] = We1.reshape(-1)[None, :]
    cst[:, 576:640] = We2.reshape(-1)[None, :]
    cst[:, 640:768] = np.arange(128, dtype=np.float32)[None, :]
    cst[:, 768:896] = np.eye(128, dtype=np.float32)

    W1lr = np.concatenate([W1l, W1r], axis=1)           # [128,128]
    W2cat = np.concatenate([W2l, W2r], axis=1)          # [64,128]
    W2lr = np.concatenate([W2cat, W2cat], axis=0)       # [128,128]

    TOT = OF_IDX + 3 * NCH
    in_maps = []
    for k in range(NCORES):
        mega = np.zeros((128, TOT), np.float32)
        mega[:, OF_XT + 0:OF_XT + PER] = x[k * PER:(k + 1) * PER].T
        mega[:, OF_CST:OF_CST + 896] = cst
        mega[:, OF_W1:OF_W1 + 128] = W1lr
        mega[:, OF_W2:OF_W2 + 128] = W2lr
        mega[:, OF_IDX + 0 * NCH:OF_IDX + 1 * NCH] = colmaj(srcg[k]).view(np.float32)
        mega[:, OF_IDX + 1 * NCH:OF_IDX + 2 * NCH] = colmaj(dstg[k])
        mega[:, OF_IDX + 2 * NCH:OF_IDX + 3 * NCH] = colmaj(wpad[k])
        in_maps.append(dict(mega=mega))
    return CG, in_maps


def _get_program(CG):
    key = ("fused", CG, _PHASE)
    if key not in _cache:
        nc = _build(CG, _PHASE)
        _cache[key] = (nc, _make_runner(nc))
    return _cache[key]


def _execute(runner, staged):
    fn = runner[0]
    out = fn(*staged)
    return out


def kernel(x, edge_index, edge_weight,
           W1l, b1l, W1r, b1r, We1, att1, bias1,
           W2l, b2l, W2r, b2r, We2, att2, bias2):
    f32 = lambda a: np.asarray(a, np.float32)
    CG, in_maps = _host_prep(
        f32(x), np.asarray(edge_index), f32(edge_weight),
        f32(W1l), f32(b1l), f32(W1r), f32(b1r), f32(We1), f32(att1), f32(bias1),
        f32(W2l), f32(b2l), f32(W2r), f32(b2r), f32(We2), f32(att2), f32(bias2))
    nc, runner = _get_program(CG)
    staged = _stage(runner, in_maps)
    out = _execute(runner, staged)
    glob = np.asarray(out[0])          # [8*PERPAD, 64]
    res = np.empty((N, 64), np.float32)
    for k in range(NCORES):
        res[k * PER:(k + 1) * PER] = glob[k * PERPAD:k * PERPAD + PER]
    return res



# revision 29
# speedup vs baseline: 1.5316x; 1.1352x over previous
"""GATv2 (2-layer) fully fused on 8 Trainium2 NeuronCores.

Design (dst-range edge sharding):
  - Nodes sharded 12500/core (padded 12544). Edges (incl. mean-fill self
    loops) sorted by dst and assigned to the core owning dst.
  - Per core: dense transforms xl/xr = x @ Wl|Wr + b on PE; AllGather of the
    per-core xl shards builds a full local xl table in each core's HBM;
    edge phase gathers xl[src] (indirect DMA from the gathered table) and
    xr[dst] (indirect DMA from the local xr table), computes GATv2 scores,
    and segment-softmax-aggregates via indicator matmuls into PSUM.
  - Softmax skips the segment-max subtraction: logits are O(30) so exp stays
    comfortably inside fp32 range, and out = (sum p*xl)/(sum p) is exact.
  - Edges are host-packed into chunks of 128; each group of 128 dst nodes
    owns CG chunks (padded with dummy edges, dstg=200 -> zero indicator row).
  - All per-core inputs ship as ONE packed [128, TOT] f32 tensor (int32
    index columns bitcast) so every consumer waits on a single DMA lane
    (walrus allows only one sync wait on a Matmult).

kernel(**inputs) -> [100000, 64] fp32.
"""
import os
import numpy as np
import jax
from jax.sharding import Mesh, PartitionSpec, NamedSharding
from jax.experimental.shard_map import shard_map

import concourse.bacc as bacc
import concourse.tile as tile
from concourse import mybir, bass
from concourse.bass2jax import (_bass_exec_p, install_neuronx_cc_hook,
                                partition_id_tensor)

F32 = mybir.dt.float32
BF16 = mybir.dt.bfloat16
I32 = mybir.dt.int32
AF = mybir.ActivationFunctionType
ALU = mybir.AluOpType

N = 100000
IN = 128
HC = 64
NCORES = 8
PER = N // NCORES            # 12500
PERPAD = 12544               # 98 * 128
G = PERPAD // 128            # 98 groups/core
NEG = 0.2

# packed-input column offsets (CG-independent part)
OF_XT = 0
OF_CST = 12544
OF_W1 = OF_CST + 896
OF_W2 = OF_W1 + 128
OF_IDX = OF_W2 + 128          # 13696; then srcg|dstg|wcol each NCH wide
# cst sub-offsets (relative to OF_CST)
B1, B2 = 0, 128
ATT1, ATT2 = 256, 320
BIA1, BIA2 = 384, 448
WE1, WE2 = 512, 576
IOTA, IDEN = 640, 768

_cache = {}
_PHASE = int(os.environ.get("K2_PHASE", "4"))


# ----------------------------------------------------------------- builder
def _build(CG, phase=4, abl=frozenset()):
    NCH = G * CG
    TOT = OF_IDX + 4 * NCH
    nc = bacc.Bacc("TRN2", target_bir_lowering=False, debug=False)
    t_mega = nc.dram_tensor("mega", [128, TOT], F32, kind="ExternalInput")
    t_out = nc.dram_tensor("out", [PERPAD, 64], F32, kind="ExternalOutput")

    with tile.TileContext(nc) as tc:
        with tc.tile_pool(name="dram", bufs=1, space="DRAM") as dpool, \
             tc.tile_pool(name="big", bufs=1) as bigp, \
             tc.tile_pool(name="sb", bufs=2) as pool, \
             tc.tile_pool(name="gat", bufs=2) as gpool, \
             tc.tile_pool(name="pd", bufs=2, space="PSUM") as psd, \
             tc.tile_pool(name="pt", bufs=2, space="PSUM") as pst, \
             tc.tile_pool(name="pz", bufs=1, space="PSUM") as psz, \
             tc.tile_pool(name="pu", bufs=2, space="PSUM") as psu:

            xl1_sh = dpool.tile([PERPAD, 64], BF16)
            xr1_tab = dpool.tile([PERPAD, 64], F32)
            xl1_tab = dpool.tile([NCORES * PERPAD, 64], BF16, addr_space="Shared")
            xl2_sh = dpool.tile([PERPAD, 64], BF16)
            xr2_tab = dpool.tile([PERPAD, 64], F32)
            xl2_tab = dpool.tile([NCORES * PERPAD, 64], BF16)

            mega = bigp.tile([128, TOT], F32)
            nc.sync.dma_start(out=mega[:], in_=t_mega[:])
            xT = mega[:, OF_XT:OF_XT + PERPAD]
            cst = mega[:, OF_CST:OF_CST + 896]
            W1 = mega[:, OF_W1:OF_W1 + 128]
            W2 = mega[:, OF_W2:OF_W2 + 128]
            srcg = mega[:, OF_IDX + 0 * NCH:OF_IDX + 1 * NCH].bitcast(I32)
            srcg2 = mega[:, OF_IDX + 1 * NCH:OF_IDX + 2 * NCH].bitcast(I32)
            dstg = mega[:, OF_IDX + 2 * NCH:OF_IDX + 3 * NCH]
            wcol = mega[:, OF_IDX + 3 * NCH:OF_IDX + 4 * NCH]
            hT = bigp.tile([128, G * 64], F32)   # tile t -> part 64*(t%2), col (t//2)*128

            # ---------------- dense 1
            for t in range(G):
                pd = psd.tile([128, 128], F32, space="PSUM", tag="pd")
                nc.tensor.matmul(pd[:], lhsT=xT[:, t * 128:(t + 1) * 128],
                                 rhs=W1[:], start=True, stop=True)
                xlb = pool.tile([128, 64], BF16, tag="xlb")
                nc.vector.tensor_add(xlb[:], pd[:, 0:64], cst[:, B1:B1 + 64])
                xrf = pool.tile([128, 64], F32, tag="xrf")
                nc.vector.tensor_add(xrf[:], pd[:, 64:128], cst[:, B1 + 64:B1 + 128])
                nc.sync.dma_start(out=xl1_sh[t * 128:(t + 1) * 128, :], in_=xlb[:])
                nc.sync.dma_start(out=xr1_tab[t * 128:(t + 1) * 128, :], in_=xrf[:])

            AGS = PERPAD // 4
            nc.gpsimd.collective_compute(
                "AllGather", ALU.bypass,
                replica_groups=[list(range(NCORES))],
                ins=[xl1_sh[:]], outs=[xl1_tab[:]])

            if phase == 1:
                for t in range(G):
                    ot = pool.tile([128, 64], F32, tag="otp1")
                    nc.sync.dma_start(out=ot[:], in_=xl1_tab[t * 128:(t + 1) * 128, :])
                    nc.sync.dma_start(out=t_out[t * 128:(t + 1) * 128, :], in_=ot[:])

            # ---------------- edge phase 1 (heads=2, c=32)
            CGW = CG * 64
            for g in range(G if phase >= 2 else 0):
                pu = psu.tile([128, 66], F32, space="PSUM", tag="pu")
                xl_G = gpool.tile([128, CGW], BF16, tag="xl", bufs=4)
                for j in range(CG):
                    c = g * CG + j
                    nc.gpsimd.indirect_dma_start(
                        out=xl_G[:, j * 64:(j + 1) * 64], out_offset=None,
                        in_=xl1_tab[:],
                        in_offset=bass.IndirectOffsetOnAxis(ap=srcg[:, c:c + 1], axis=0))
                xr_grp = gpool.tile([128, 64], F32, tag="xr")
                nc.sync.dma_start(out=xr_grp[:], in_=xr1_tab[g * 128:(g + 1) * 128, :])
                # z0 = xl + We1*w  (group-wide)
                t1G = pool.tile([128, CGW], F32, tag="t1")
                nc.vector.tensor_tensor(
                    out=t1G.rearrange("p (j c) -> p j c", c=64),
                    in0=cst[:, WE1:WE1 + 64].rearrange("p (o c) -> p o c", o=1)
                        .to_broadcast([128, CG, 64]),
                    in1=wcol[:, g * CG:(g + 1) * CG].to_broadcast([128, CG, 64]),
                    op=ALU.mult)
                z0G = pool.tile([128, CGW], F32, tag="z0")
                nc.vector.tensor_add(z0G[:], xl_G[:], t1G[:])
                indG = pool.tile([128, CG * 128], F32, tag="ind")
                for j in range(CG):
                    c = g * CG + j
                    nc.vector.tensor_tensor(
                        out=indG[:, j * 128:(j + 1) * 128],
                        in0=dstg[:, c:c + 1].to_broadcast([128, 128]),
                        in1=cst[:, IOTA:IOTA + 128], op=ALU.is_equal)
                pzG = psz.tile([128, CGW], F32, space="PSUM", tag="pz")
                for j in range(CG):
                    ptt = pst.tile([128, 128], F32, space="PSUM", tag="ptt")
                    nc.tensor.transpose(out=ptt[:], in_=indG[:, j * 128:(j + 1) * 128],
                                        identity=cst[:, IDEN:IDEN + 128])
                    indT = pool.tile([128, 128], F32, tag="indT", bufs=3)
                    nc.vector.tensor_copy(indT[:], ptt[:])
                    nc.tensor.matmul(pzG[:, j * 64:(j + 1) * 64], lhsT=indT[:],
                                     rhs=xr_grp[:], start=True, stop=True)
                zG = pool.tile([128, CGW], F32, tag="z")
                nc.vector.tensor_add(zG[:], z0G[:], pzG[:])
                lrG = pool.tile([128, CGW], F32, tag="lr")
                nc.scalar.activation(lrG[:], zG[:], AF.Prelu, alpha=NEG)
                lrwG = pool.tile([128, CGW], F32, tag="lrw")
                nc.vector.tensor_tensor(
                    out=lrwG.rearrange("p (j c) -> p j c", c=64),
                    in0=lrG.rearrange("p (j c) -> p j c", c=64),
                    in1=cst[:, ATT1:ATT1 + 64].rearrange("p (o c) -> p o c", o=1)
                        .to_broadcast([128, CG, 64]),
                    op=ALU.mult)
                laG = pool.tile([128, CG * 2], F32, tag="la")
                nc.vector.tensor_reduce(
                    out=laG[:], in_=lrwG.rearrange("p (a c) -> p a c", c=32),
                    axis=mybir.AxisListType.X, op=ALU.add)
                vtG = pool.tile([128, CG * 66], F32, tag="vt")
                vt3 = vtG.rearrange("p (j k) -> p j k", k=66)
                nc.scalar.activation(vt3[:, :, 64:66],
                                     laG.rearrange("p (j h) -> p j h", h=2), AF.Exp)
                xl3 = xl_G.rearrange("p (j c) -> p j c", c=64)
                for h in range(2):
                    nc.vector.tensor_tensor(
                        out=vt3[:, :, h * 32:(h + 1) * 32],
                        in0=xl3[:, :, h * 32:(h + 1) * 32],
                        in1=vt3[:, :, 64 + h:65 + h].to_broadcast([128, CG, 32]),
                        op=ALU.mult)
                for j in range(CG):
                    nc.tensor.matmul(pu[:], lhsT=indG[:, j * 128:(j + 1) * 128],
                                     rhs=vtG[:, j * 66:(j + 1) * 66],
                                     start=(j == 0), stop=(j == CG - 1))
                # finalize group: h = relu(u/s + bias1), store transposed
                # (clamp s away from 0 so empty padding rows give 0, not NaN)
                sm = pool.tile([128, 2], F32, tag="sm")
                nc.vector.tensor_scalar_max(sm[:], pu[:, 64:66], 1e-30)
                rec = pool.tile([128, 2], F32, tag="rec")
                nc.vector.reciprocal(rec[:], sm[:])
                h = pool.tile([128, 64], F32, tag="h")
                nc.vector.tensor_mul(h[:, 0:32], pu[:, 0:32],
                                     rec[:, 0:1].to_broadcast([128, 32]))
                nc.vector.tensor_mul(h[:, 32:64], pu[:, 32:64],
                                     rec[:, 1:2].to_broadcast([128, 32]))
                nc.vector.tensor_add(h[:], h[:], cst[:, BIA1:BIA1 + 64])
                hr = pool.tile([128, 64], F32, tag="hr")
                nc.vector.tensor_scalar_max(hr[:], h[:], 0.0)
                ptt = pst.tile([64, 128], F32, space="PSUM", tag="ptt")
                nc.tensor.transpose(out=ptt[:], in_=hr[:], identity=cst[:, IDEN:IDEN + 128])
                po = 64 * (g % 2)
                nc.vector.tensor_copy(hT[po:po + 64, (g // 2) * 128:(g // 2) * 128 + 128],
                                      ptt[:])
                if phase == 2:
                    nc.sync.dma_start(out=t_out[g * 128:(g + 1) * 128, :], in_=hr[:])

            # ---------------- dense 2
            for t in range(G if phase >= 3 else 0):
                po = 64 * (t % 2)
                pd = psd.tile([128, 128], F32, space="PSUM", tag="pd")
                nc.tensor.matmul(pd[:], lhsT=hT[po:po + 64, (t // 2) * 128:(t // 2) * 128 + 128],
                                 rhs=W2[po:po + 64, :], start=True, stop=True)
                xlb = pool.tile([128, 64], BF16, tag="xlb")
                nc.vector.tensor_add(xlb[:], pd[:, 0:64], cst[:, B2:B2 + 64])
                xrf = pool.tile([128, 64], F32, tag="xrf")
                nc.vector.tensor_add(xrf[:], pd[:, 64:128], cst[:, B2 + 64:B2 + 128])
                nc.sync.dma_start(out=xl2_sh[t * 128:(t + 1) * 128, :], in_=xlb[:])
                nc.sync.dma_start(out=xr2_tab[t * 128:(t + 1) * 128, :], in_=xrf[:])

            if phase == 31:
                for t in range(G):
                    ot = pool.tile([128, 64], F32, tag="otp3")
                    nc.sync.dma_start(out=ot[:], in_=xl2_sh[t * 128:(t + 1) * 128, :])
                    nc.sync.dma_start(out=t_out[t * 128:(t + 1) * 128, :], in_=ot[:])

            if phase >= 3 and phase != 31:
                for k in range(4):
                    nc.gpsimd.collective_compute(
                        "AllGather", ALU.bypass,
                        replica_groups=[list(range(NCORES))],
                        ins=[xl2_sh[k * AGS:(k + 1) * AGS, :]],
                        outs=[xl2_tab[k * NCORES * AGS:(k + 1) * NCORES * AGS, :]])

            if phase == 3:
                for t in range(G):
                    ot = pool.tile([128, 64], F32, tag="otp3")
                    nc.sync.dma_start(out=ot[:], in_=xl2_tab[t * 128:(t + 1) * 128, :])
                    nc.sync.dma_start(out=t_out[t * 128:(t + 1) * 128, :], in_=ot[:])

            # ---------------- edge phase 2 (heads=1, c=64)
            for g in range(G if phase >= 4 and phase != 31 else 0):
                pu = psu.tile([128, 65], F32, space="PSUM", tag="pu")
                xl_G = gpool.tile([128, CGW], BF16, tag="xl2", bufs=4)
                for j in range(CG):
                    c = g * CG + j
                    nc.gpsimd.indirect_dma_start(
                        out=xl_G[:, j * 64:(j + 1) * 64], out_offset=None,
                        in_=xl2_tab[:],
                        in_offset=bass.IndirectOffsetOnAxis(ap=srcg2[:, c:c + 1], axis=0))
                xr_grp = gpool.tile([128, 64], F32, tag="xr2")
                nc.sync.dma_start(out=xr_grp[:], in_=xr2_tab[g * 128:(g + 1) * 128, :])
                t1G = pool.tile([128, CGW], F32, tag="t12")
                nc.vector.tensor_tensor(
                    out=t1G.rearrange("p (j c) -> p j c", c=64),
                    in0=cst[:, WE2:WE2 + 64].rearrange("p (o c) -> p o c", o=1)
                        .to_broadcast([128, CG, 64]),
                    in1=wcol[:, g * CG:(g + 1) * CG].to_broadcast([128, CG, 64]),
                    op=ALU.mult)
                z0G = pool.tile([128, CGW], F32, tag="z02")
                nc.vector.tensor_add(z0G[:], xl_G[:], t1G[:])
                indG = pool.tile([128, CG * 128], F32, tag="ind2")
                for j in range(CG):
                    c = g * CG + j
                    nc.vector.tensor_tensor(
                        out=indG[:, j * 128:(j + 1) * 128],
                        in0=dstg[:, c:c + 1].to_broadcast([128, 128]),
                        in1=cst[:, IOTA:IOTA + 128], op=ALU.is_equal)
                pzG = psz.tile([128, CGW], F32, space="PSUM", tag="pz")
                for j in range(CG):
                    ptt = pst.tile([128, 128], F32, space="PSUM", tag="ptt")
                    nc.tensor.transpose(out=ptt[:], in_=indG[:, j * 128:(j + 1) * 128],
                                        identity=cst[:, IDEN:IDEN + 128])
                    indT = pool.tile([128, 128], F32, tag="indT", bufs=3)
                    nc.vector.tensor_copy(indT[:], ptt[:])
                    nc.tensor.matmul(pzG[:, j * 64:(j + 1) * 64], lhsT=indT[:],
                                     rhs=xr_grp[:], start=True, stop=True)
                zG = pool.tile([128, CGW], F32, tag="z2")
                nc.vector.tensor_add(zG[:], z0G[:], pzG[:])
                lrG = pool.tile([128, CGW], F32, tag="lr2")
                nc.scalar.activation(lrG[:], zG[:], AF.Prelu, alpha=NEG)
                lrwG = pool.tile([128, CGW], F32, tag="lrw2")
                nc.vector.tensor_tensor(
                    out=lrwG.rearrange("p (j c) -> p j c", c=64),
                    in0=lrG.rearrange("p (j c) -> p j c", c=64),
                    in1=cst[:, ATT2:ATT2 + 64].rearrange("p (o c) -> p o c", o=1)
                        .to_broadcast([128, CG, 64]),
                    op=ALU.mult)
                laG = pool.tile([128, CG], F32, tag="la2")
                nc.vector.tensor_reduce(
                    out=laG[:], in_=lrwG.rearrange("p (a c) -> p a c", c=64),
                    axis=mybir.AxisListType.X, op=ALU.add)
                vtG = pool.tile([128, CG * 65], F32, tag="vt2")
                vt3 = vtG.rearrange("p (j k) -> p j k", k=65)
                nc.scalar.activation(vt3[:, :, 64:65],
                                     laG.rearrange("p (j h) -> p j h", h=1), AF.Exp)
                xl3 = xl_G.rearrange("p (j c) -> p j c", c=64)
                nc.vector.tensor_tensor(
                    out=vt3[:, :, 0:64], in0=xl3[:, :, :],
                    in1=vt3[:, :, 64:65].to_broadcast([128, CG, 64]),
                    op=ALU.mult)
                for j in range(CG):
                    nc.tensor.matmul(pu[:], lhsT=indG[:, j * 128:(j + 1) * 128],
                                     rhs=vtG[:, j * 65:(j + 1) * 65],
                                     start=(j == 0), stop=(j == CG - 1))
                sm = pool.tile([128, 1], F32, tag="sm2")
                nc.vector.tensor_scalar_max(sm[:], pu[:, 64:65], 1e-30)
                rec = pool.tile([128, 1], F32, tag="rec2")
                nc.vector.reciprocal(rec[:], sm[:])
                o = pool.tile([128, 64], F32, tag="o")
                nc.vector.tensor_mul(o[:], pu[:, 0:64], rec[:].to_broadcast([128, 64]))
                nc.vector.tensor_add(o[:], o[:], cst[:, BIA2:BIA2 + 64])
                nc.sync.dma_start(out=t_out[g * 128:(g + 1) * 128, :], in_=o[:])
    nc.compile()
    return nc


# ----------------------------------------------------------------- runner
def _make_runner(nc):
    install_neuronx_cc_hook()
    in_names, out_names, out_avals = [], [], []
    partition_name = nc.partition_id_tensor.name if nc.partition_id_tensor else None
    for alloc in nc.m.functions[0].allocations:
        if not isinstance(alloc, mybir.MemoryLocationSet):
            continue
        name = alloc.memorylocations[0].name
        if alloc.kind == "ExternalInput":
            if name != partition_name:
                in_names.append(name)
        elif alloc.kind == "ExternalOutput":
            out_names.append(name)
            out_avals.append(jax.core.ShapedArray(tuple(alloc.tensor_shape),
                                                  mybir.dt.np(alloc.dtype)))
    n_params = len(in_names)
    n_outs = len(out_avals)
    all_in_names = list(in_names) + list(out_names)
    if partition_name is not None:
        all_in_names.append(partition_name)

    def _body(*args):
        operands = list(args)
        if partition_name is not None:
            operands.append(partition_id_tensor())
        outs = _bass_exec_p.bind(
            *operands,
            out_avals=tuple(out_avals),
            in_names=tuple(all_in_names),
            out_names=tuple(out_names),
            lowering_input_output_aliases=(),
            sim_require_finite=True,
            sim_require_nnan=True,
            nc=nc,
        )
        return tuple(outs)

    devices = jax.devices()[:NCORES]
    mesh = Mesh(np.asarray(devices), ("core",))
    in_specs = (PartitionSpec("core"),) * (n_params + n_outs)
    out_specs = (PartitionSpec("core"),) * n_outs
    fn = jax.jit(shard_map(_body, mesh=mesh, in_specs=in_specs,
                           out_specs=out_specs, check_rep=False),
                 keep_unused=True)
    return fn, in_names, out_names, out_avals, mesh, devices


def _stage(runner, in_maps):
    """Device-put per-core inputs (plus zero output feeds) as sharded arrays."""
    fn, in_names, out_names, out_avals, mesh, devices = runner
    staged = []
    for name in in_names:
        shards = [jax.device_put(np.ascontiguousarray(in_maps[k][name]), devices[k])
                  for k in range(NCORES)]
        jax.block_until_ready(shards)
        shp = in_maps[0][name].shape
        arr = jax.make_array_from_single_device_arrays(
            (NCORES * shp[0],) + tuple(shp[1:]),
            NamedSharding(mesh, PartitionSpec("core")), shards)
        staged.append(arr)
    for av in out_avals:
        z = np.zeros(av.shape, av.dtype)
        shards = [jax.device_put(z, d) for d in devices]
        jax.block_until_ready(shards)
        arr = jax.make_array_from_single_device_arrays(
            (NCORES * av.shape[0],) + tuple(av.shape[1:]),
            NamedSharding(mesh, PartitionSpec("core")), shards)
        staged.append(arr)
    return staged


# ----------------------------------------------------------------- host prep
_PERM = None


def _balance_perm(dst):
    """Degree-balanced node renumbering: pack nodes into (core, group) bins so
    every 128-node dst group has nearly equal in-degree (minimizes CG)."""
    import heapq
    deg = np.bincount(dst, minlength=N)
    order = np.argsort(-deg, kind="stable")
    NB = NCORES * G
    cap = np.full(NB, 128, np.int64)
    cap[G - 1::G] = PER - (G - 1) * 128          # 84: last group per core
    load = np.zeros(NB, np.int64)
    cnt = np.zeros(NB, np.int64)
    heap = [(0, b) for b in range(NB)]
    heapq.heapify(heap)
    perm = np.empty(N, np.int64)
    for v in order:
        while True:
            l, b = heapq.heappop(heap)
            if cnt[b] < cap[b]:
                break
        perm[v] = (b // G) * PER + (b % G) * 128 + cnt[b]
        cnt[b] += 1
        load[b] += deg[v]
        if cnt[b] < cap[b]:
            heapq.heappush(heap, (load[b], b))
    return perm


def _host_prep(x, edge_index, edge_weight,
               W1l, b1l, W1r, b1r, We1, att1, bias1,
               W2l, b2l, W2r, b2r, We2, att2, bias2):
    global _PERM
    src = edge_index[0].astype(np.int64)
    dst = edge_index[1].astype(np.int64)
    _PERM = _balance_perm(dst)
    src = _PERM[src]
    dst = _PERM[dst]
    xp = np.empty_like(x)
    xp[_PERM] = x
    x = xp
    ew = edge_weight[:, 0].astype(np.float64)
    deg = np.bincount(dst, minlength=N).astype(np.float64)
    wsum = np.bincount(dst, weights=ew, minlength=N)
    loop_w = (wsum / np.maximum(deg, 1.0)).astype(np.float32)

    allsrc = np.concatenate([src, np.arange(N, dtype=np.int64)])
    alldst = np.concatenate([dst, np.arange(N, dtype=np.int64)])
    allw = np.concatenate([edge_weight[:, 0].astype(np.float32), loop_w])
    order = np.argsort(alldst, kind="stable")
    ss, ds, ws = allsrc[order], alldst[order], allw[order]

    core = ds // PER
    loc = ds % PER
    gid = core * G + loc // 128
    gcnt = np.bincount(gid, minlength=NCORES * G)
    CG = max(2, int(np.ceil(gcnt.max() / 128.0)))
    NCH = G * CG
    gstart = np.zeros(NCORES * G + 1, np.int64)
    np.cumsum(gcnt, out=gstart[1:])
    slot = np.arange(ds.shape[0], dtype=np.int64) - gstart[gid]

    SLOTS = NCH * 128
    srcg = np.zeros((NCORES, SLOTS), np.int32)
    srcg2 = np.zeros((NCORES, SLOTS), np.int32)
    dstl = np.zeros((NCORES, SLOTS), np.int32)
    dstg = np.full((NCORES, SLOTS), 200.0, np.float32)
    wpad = np.zeros((NCORES, SLOTS), np.float32)
    ce = gid // G
    pos = (gid % G) * (CG * 128) + slot
    srcg[ce, pos] = ((ss // PER) * PERPAD + ss % PER).astype(np.int32)
    AGS = PERPAD // 4
    sc, sl = ss // PER, ss % PER
    srcg2[ce, pos] = ((sl // AGS * NCORES + sc) * AGS + sl % AGS).astype(np.int32)
    dstl[ce, pos] = loc.astype(np.int32)
    dstg[ce, pos] = (loc % 128).astype(np.float32)
    wpad[ce, pos] = ws

    def colmaj(a):  # [SLOTS] -> [128, NCH]
        return np.ascontiguousarray(a.reshape(NCH, 128).T)

    cst = np.zeros((128, 896), np.float32)
    cst[:, 0:128] = np.concatenate([b1l, b1r])[None, :]
    cst[:, 128:256] = np.concatenate([b2l, b2r])[None, :]
    cst[:, 256:320] = att1.reshape(-1)[None, :]
    cst[:, 320:384] = att2.reshape(-1)[None, :]
    cst[:, 384:448] = bias1[None, :]
    cst[:, 448:512] = bias2[None, :]
    cst[:, 512:576] = We1.reshape(-1)[None, :]
    cst[:, 576:640] = We2.reshape(-1)[None, :]
    cst[:, 640:768] = np.arange(128, dtype=np.float32)[None, :]
    cst[:, 768:896] = np.eye(128, dtype=np.float32)

    W1lr = np.concatenate([W1l, W1r], axis=1)           # [128,128]
    W2cat = np.concatenate([W2l, W2r], axis=1)          # [64,128]
    W2lr = np.concatenate([W2cat, W2cat], axis=0)       # [128,128]

    TOT = OF_IDX + 4 * NCH
    in_maps = []
    for k in range(NCORES):
        mega = np.zeros((128, TOT), np.float32)
        mega[:, OF_XT + 0:OF_XT + PER] = x[k * PER:(k + 1) * PER].T
        mega[:, OF_CST:OF_CST + 896] = cst
        mega[:, OF_W1:OF_W1 + 128] = W1lr
        mega[:, OF_W2:OF_W2 + 128] = W2lr
        mega[:, OF_IDX + 0 * NCH:OF_IDX + 1 * NCH] = colmaj(srcg[k]).view(np.float32)
        mega[:, OF_IDX + 1 * NCH:OF_IDX + 2 * NCH] = colmaj(srcg2[k]).view(np.float32)
        mega[:, OF_IDX + 2 * NCH:OF_IDX + 3 * NCH] = colmaj(dstg[k])
        mega[:, OF_IDX + 3 * NCH:OF_IDX + 4 * NCH] = colmaj(wpad[k])
        in_maps.append(dict(mega=mega))
    return CG, in_maps


def _get_program(CG):
    key = ("fused", CG, _PHASE)
    if key not in _cache:
        nc = _build(CG, _PHASE)
        _cache[key] = (nc, _make_runner(nc))
    return _cache[key]


def _execute(runner, staged):
    fn = runner[0]
    out = fn(*staged)
    return out


def kernel(x, edge_index, edge_weight,
           W1l, b1l, W1r, b1r, We1, att1, bias1,
           W2l, b2l, W2r, b2r, We2, att2, bias2):
    f32 = lambda a: np.asarray(a, np.float32)
    CG, in_maps = _host_prep(
        f32(x), np.asarray(edge_index), f32(edge_weight),
        f32(W1l), f32(b1l), f32(W1r), f32(b1r), f32(We1), f32(att1), f32(bias1),
        f32(W2l), f32(b2l), f32(W2r), f32(b2r), f32(We2), f32(att2), f32(bias2))
    nc, runner = _get_program(CG)
    staged = _stage(runner, in_maps)
    out = _execute(runner, staged)
    glob = np.asarray(out[0])          # [8*PERPAD, 64]
    res = np.empty((N, 64), np.float32)
    for k in range(NCORES):
        res[k * PER:(k + 1) * PER] = glob[k * PERPAD:k * PERPAD + PER]
    return res[_PERM]

